# revision 1
# baseline (speedup 1.0000x reference)
"""Causal self-attention with RoPE on 8 Trainium2 NeuronCores.

Full inputs: x [4, 2048, 1024], W_attn [1024, 3072], W_proj [1024, 1024] (f32).
Sharding: core = b*2 + hg  (b in 0..3 batches, hg in 0..1 head-groups of 8 heads).
Each core computes qkv for its 8 heads, attention, and a partial output
projection (row-parallel c_proj); host sums the two partials per batch.

All matmuls run in float32r (fp32 with 11-bit mantissa, full PE rate at
N>=256). The PE rounds raw fp32 operands itself; on-chip producers of matmul
operands (DVE/ACT) write float32r-typed tiles to satisfy the BIR verifier.
"""

import sys

sys.path.insert(0, "/opt/trn_rl_repo")

import numpy as np

import concourse.bass as bass  # noqa: F401
import concourse.mybir as mybir
import concourse.tile as tile
from concourse import bacc
from concourse.bass_utils import run_bass_kernel_spmd

F32 = mybir.dt.float32
F32R = mybir.dt.float32r
AF = mybir.ActivationFunctionType
OP = mybir.AluOpType

B, T, C = 4, 2048, 1024
H, D = 16, 64
HPC = 8            # heads per core
CO_QKV = 3 * HPC * D   # 1536 qkv columns per core
NEG = -30000.0     # additive mask; exp(S + NEG) == 0 exactly on ACT

TC = 512           # t-chunk width in phase A
N_TC = T // TC     # 8
N_KO = C // 128    # 8 contraction chunks
N_CT = 2 * HPC * D // 128   # 8 q+k column tiles (4 q, 4 k)
N_TT = T // 128    # 16 t tiles
N_IC = 4           # i-chunks of 512 queries
IC = 512


def round_fp32r(x):
    b = np.ascontiguousarray(x, dtype=np.float32).view(np.uint32)
    r = ((b.astype(np.uint64) + 0x800) & 0xFFFFF000).astype(np.uint32)
    return r.view(np.float32).reshape(x.shape)


def _rope_tables():
    """cosT/sinN [128, T] f32: row p holds freq for d = p % 64; sinN has the
    rotate-half sign folded in (rows d<32 negative)."""
    inv_freq = (
        np.float32(1.0)
        / np.float32(10000.0) ** (np.arange(0, D, 2, dtype=np.float32) / np.float32(D))
    ).astype(np.float32)
    t = np.arange(T, dtype=np.float32)
    freqs = (t[:, None] * inv_freq[None, :]).astype(np.float32)  # [T, 32]
    emb = np.concatenate([freqs, freqs], axis=1)  # [T, 64]
    cos = np.cos(emb).astype(np.float32)
    sin = np.sin(emb).astype(np.float32)
    sinN = np.concatenate([-sin[:, :32], sin[:, 32:]], axis=1)
    cosT = np.tile(cos.T, (2, 1))   # [128, T]
    sinNT = np.tile(sinN.T, (2, 1))
    return np.ascontiguousarray(cosT), np.ascontiguousarray(sinNT)


def _mask_table():
    """tril01 [128, 128]: 1.0 if j <= c else 0.0."""
    j = np.arange(128)[:, None]
    c = np.arange(128)[None, :]
    return (j <= c).astype(np.float32)


def _build():
    nc = bacc.Bacc(None, target_bir_lowering=False, debug=False)

    xT = nc.dram_tensor("xT", [C, T], F32R, kind="ExternalInput")
    wqkv = nc.dram_tensor("wqkv", [C, CO_QKV], F32R, kind="ExternalInput")
    wproj = nc.dram_tensor("wproj", [HPC * D, C], F32R, kind="ExternalInput")
    cosT_d = nc.dram_tensor("cosT", [128, T], F32, kind="ExternalInput")
    sinN_d = nc.dram_tensor("sinN", [128, T], F32, kind="ExternalInput")
    tril_d = nc.dram_tensor("tril01", [128, 128], F32, kind="ExternalInput")
    ones_d = nc.dram_tensor("ones", [128, 64], F32R, kind="ExternalInput")
    out_d = nc.dram_tensor("out", [T, C], F32, kind="ExternalOutput")

    xT_r = xT.rearrange("(ko p) t -> p ko t", p=128)
    wqkv_r = wqkv.rearrange("(ko p) c -> p ko c", p=128)
    wproj_r = wproj.rearrange("(ko p) c -> p ko c", p=128)

    with tile.TileContext(nc) as tc:
        with (
            tc.tile_pool(name="resident", bufs=1) as res,
            tc.tile_pool(name="qkv", bufs=1) as qkv_pool,
        ):
            # ---- resident tables + outputs of phase A ----
            cos_sb = res.tile([128, T], F32)
            sinN_sb = res.tile([128, T], F32)
            tril_sb = res.tile([128, 128], F32)
            ones_sb = res.tile([128, 64], F32R)

            # q^T / k^T: [p = d within head-pair, hp, t]
            qT = qkv_pool.tile([128, HPC // 2, T], F32R)
            kT = qkv_pool.tile([128, HPC // 2, T], F32R)
            # v: [p = t%128, t//128, head, 65] with ones column at d=64
            v_sb = qkv_pool.tile([128, N_TT, HPC, D + 1], F32R)
            # ---------------- Phase A: QKV + RoPE ----------------
            with (
                tc.tile_pool(name="w", bufs=1) as wp,
                tc.tile_pool(name="xt", bufs=2) as xtp,
                tc.tile_pool(name="rope", bufs=5) as ropep,
                tc.tile_pool(name="ps_qk", bufs=5, space="PSUM") as ps_qk,
                tc.tile_pool(name="ps_v", bufs=3, space="PSUM") as ps_v,
            ):
                w_sb = wp.tile([128, N_KO, CO_QKV], F32R)
                # first compute's inputs first: xt chunk 0, then q/k weights,
                # then tables, then v weights
                xt0 = xtp.tile([128, N_KO, TC], F32R, name="xt0", tag="xt")
                # interleave per-ko pieces of xt chunk 0 and W column tile 0
                # so the first accumulation step's inputs arrive first
                for ko in range(N_KO):
                    nc.sync.dma_start(xt0[:, ko], xT_r[:, ko, 0:TC])
                    nc.sync.dma_start(w_sb[:, ko, 0:128], wqkv_r[:, ko, 0:128])
                for wct in range(1, 8):
                    nc.sync.dma_start(
                        w_sb[:, :, wct * 128 : (wct + 1) * 128],
                        wqkv_r[:, :, wct * 128 : (wct + 1) * 128],
                    )
                nc.sync.dma_start(cos_sb[:], cosT_d[:])
                nc.sync.dma_start(sinN_sb[:], sinN_d[:])
                nc.sync.dma_start(tril_sb[:], tril_d[:])
                nc.sync.dma_start(ones_sb[:], ones_d[:])
                for wct in range(8, CO_QKV // 128):
                    nc.sync.dma_start(
                        w_sb[:, :, wct * 128 : (wct + 1) * 128],
                        wqkv_r[:, :, wct * 128 : (wct + 1) * 128],
                    )

                # ones columns of v (written once)
                for to in range(N_TT):
                    nc.gpsimd.tensor_copy(v_sb[:, to, :, D], ones_sb[:, 0:HPC])

                for tc_i in range(N_TC):
                    ts_ = slice(tc_i * TC, (tc_i + 1) * TC)
                    if tc_i == 0:
                        xt_sb = xt0
                    else:
                        xt_sb = xtp.tile([128, N_KO, TC], F32R, name="xt", tag="xt")
                        for ko in range(N_KO):
                            nc.sync.dma_start(xt_sb[:, ko], xT_r[:, ko, ts_])

                    # q and k column tiles (ct 0-3 -> q head-pair, 4-7 -> k)
                    for ct in range(N_CT):
                        psum = ps_qk.tile([128, TC], F32)
                        for ko in range(N_KO):
                            nc.tensor.matmul(
                                psum[:],
                                w_sb[:, ko, ct * 128 : (ct + 1) * 128],
                                xt_sb[:, ko, :],
                                start=(ko == 0),
                                stop=(ko == N_KO - 1),
                            )
                        hp = ct % 4
                        dest = (qT if ct < 4 else kT)[:, hp, ts_]
                        # RoPE: dest = psum * cos + shift(psum) * sinN
                        rot = ropep.tile([128, TC], F32)
                        for blk in range(4):
                            src = (blk ^ 1) * 32
                            if blk < 2:
                                nc.vector.tensor_copy(
                                    rot[blk * 32 : blk * 32 + 32, :],
                                    psum[src : src + 32, :],
                                )
                            else:
                                nc.scalar.copy(
                                    rot[blk * 32 : blk * 32 + 32, :],
                                    psum[src : src + 32, :],
                                )
                        nc.gpsimd.tensor_tensor(rot[:], rot[:], sinN_sb[:, ts_], OP.mult)
                        nc.vector.tensor_tensor(dest, psum[:], cos_sb[:, ts_], OP.mult)
                        nc.gpsimd.tensor_tensor(dest, dest, rot[:], OP.add)

                    # v tiles: two 128-row t sub-tiles per chunk
                    for sub in range(TC // 128):
                        to = tc_i * (TC // 128) + sub
                        psv = ps_v.tile([128, HPC * D], F32)
                        for ko in range(N_KO):
                            nc.tensor.matmul(
                                psv[:],
                                xt_sb[:, ko, sub * 128 : sub * 128 + 128],
                                w_sb[:, ko, 2 * HPC * D : 3 * HPC * D],
                                start=(ko == 0),
                                stop=(ko == N_KO - 1),
                            )
                        nc.scalar.copy(
                            v_sb[:, to, :, 0:D],
                            psv[:].rearrange("p (h d) -> p h d", d=D),
                        )

            # ---------------- Phases B+C pool (opens after phase A frees W/xT) ----
            with tc.tile_pool(name="yt", bufs=1) as ytp:
                # y^T: same layout as qT
                yT = ytp.tile([128, HPC // 2, T], F32R)
                _phase_bc(nc, tc, qT, kT, v_sb, yT, tril_sb, ones_sb, wproj_r, out_d)

    nc.compile()
    return nc


def _emit_av(nc, ypsum, v_sb, hp, ic, n_jt, jt, expT, lo):
    for hl in range(2):
        nc.tensor.matmul(
            ypsum[hl][:, lo:],
            v_sb[:, jt, 2 * hp + hl, :],
            expT[:, hl, lo:],
            start=(jt == 0),
            stop=(jt == n_jt - 1),
        )


def _phase_bc(nc, tc, qT, kT, v_sb, yT, tril_sb, ones_sb, wproj_r, out_d):
    """Attention + fused output projection, i-chunk major so the projection of
    chunk ic overlaps attention of chunk ic+1."""
    with (
        tc.tile_pool(name="exp", bufs=6) as expp,
        tc.tile_pool(name="fin", bufs=2) as finp,
        tc.tile_pool(name="wp2", bufs=1) as wp2,
        tc.tile_pool(name="ostage", bufs=3) as osp,
        tc.tile_pool(name="ps_s", bufs=4, space="PSUM") as ps_s,
        tc.tile_pool(name="ps_y", bufs=2, space="PSUM") as ps_y,
        tc.tile_pool(name="ps_o", bufs=1, space="PSUM") as ps_o,
    ):
        wp_sb = wp2.tile([128, 4, C], F32R)
        nc.sync.dma_start(wp_sb[:], wproj_r[:])
        for ic in range(N_IC):
            is_ = slice(ic * IC, (ic + 1) * IC)
            n_jt = (ic + 1) * 4
            for hp in range(HPC // 2):
                ypsum = [
                    ps_y.tile([D + 1, IC], F32, name=f"ypsum{_hl}", tag="ypsum")
                    for _hl in range(2)
                ]
                pending = None
                for jt in range(n_jt):
                    k_diag = jt - ic * 4
                    lo = max(0, k_diag * 128)
                    expT = expp.tile([128, 2, IC], F32R)
                    for hl in range(2):
                        pb = hl * 64
                        sps = ps_s.tile([128, IC], F32)
                        nc.tensor.matmul(
                            sps[:, lo:],
                            kT[pb : pb + 64, hp, jt * 128 : (jt + 1) * 128],
                            qT[pb : pb + 64, hp, ic * IC + lo : (ic + 1) * IC],
                            start=True,
                            stop=True,
                        )
                        nc.scalar.activation(expT[:, hl, lo:], sps[:, lo:], AF.Exp)
                    if k_diag >= 0:
                        # zero the upper-triangular part of the boundary blocks
                        for hl in range(2):
                            nc.gpsimd.tensor_tensor(
                                expT[:, hl, lo : lo + 128],
                                expT[:, hl, lo : lo + 128],
                                tril_sb[:],
                                OP.mult,
                            )
                    if pending is not None:
                        _emit_av(nc, ypsum, v_sb, hp, ic, n_jt, *pending)
                    pending = (jt, expT, lo)
                _emit_av(nc, ypsum, v_sb, hp, ic, n_jt, *pending)
                for hl in range(2):
                    pb = hl * 64
                    recip = finp.tile([1, IC], F32R)
                    with nc.allow_low_precision(reason="softmax recip f32r"):
                        nc.vector.reciprocal(recip[:], ypsum[hl][D : D + 1, :])
                    bc = ps_o.tile([D, IC], F32, name="bc", tag="bc")
                    nc.tensor.matmul(
                        bc[:], ones_sb[0:1, 0:D], recip[:], start=True, stop=True
                    )
                    bc_sb = finp.tile([D, IC], F32, name="bc_sb", tag="bc_sb")
                    nc.vector.tensor_copy(bc_sb[:], bc[:])
                    nc.vector.tensor_tensor(
                        yT[pb : pb + 64, hp, is_],
                        ypsum[hl][0:D, :],
                        bc_sb[:],
                        OP.mult,
                    )
            # output projection for the t-tiles this i-chunk completed
            for tt in range(ic * 4, (ic + 1) * 4):
                for cc in range(C // 512):
                    po = ps_o.tile([128, 512], F32)
                    for ko in range(4):
                        nc.tensor.matmul(
                            po[:],
                            yT[:, ko, tt * 128 : (tt + 1) * 128],
                            wp_sb[:, ko, cc * 512 : (cc + 1) * 512],
                            start=(ko == 0),
                            stop=(ko == 3),
                        )
                    ost = osp.tile([128, 512], F32)
                    nc.scalar.copy(ost[:], po[:])
                    nc.sync.dma_start(
                        out_d[tt * 128 : (tt + 1) * 128, cc * 512 : (cc + 1) * 512],
                        ost[:],
                    )


_NC = None
_STATE = None


def _get_nc():
    global _NC
    if _NC is None:
        _NC = _build()
    return _NC


def _get_state():
    """Build the bass module once and cache a persistent jitted executor plus
    device-resident static tables (rope/mask/ones are pure functions of the
    problem shape)."""
    global _STATE
    if _STATE is not None:
        return _STATE

    import jax
    import jax.numpy as jnp
    from jax.experimental.shard_map import shard_map
    from jax.sharding import Mesh, NamedSharding, PartitionSpec

    from concourse import bass2jax

    nc = _get_nc()
    bass2jax.install_neuronx_cc_hook()
    partition_name = nc.partition_id_tensor.name if nc.partition_id_tensor else None
    in_names, out_names, out_avals = [], [], []
    for alloc in nc.m.functions[0].allocations:
        if not isinstance(alloc, mybir.MemoryLocationSet):
            continue
        name = alloc.memorylocations[0].name
        if alloc.kind == "ExternalInput":
            if name != partition_name:
                in_names.append(name)
        elif alloc.kind == "ExternalOutput":
            out_names.append(name)
            out_avals.append(
                jax.core.ShapedArray(tuple(alloc.tensor_shape), mybir.dt.np(alloc.dtype))
            )
    n_params, n_outs = len(in_names), len(out_avals)
    all_names = list(in_names) + out_names
    if partition_name:
        all_names.append(partition_name)

    def _body(*args):
        operands = list(args)
        if partition_name:
            operands.append(bass2jax.partition_id_tensor())
        outs = bass2jax._bass_exec_p.bind(
            *operands,
            out_avals=tuple(out_avals),
            in_names=tuple(all_names),
            out_names=tuple(out_names),
            lowering_input_output_aliases=(),
            sim_require_finite=True,
            sim_require_nnan=True,
            nc=nc,
        )
        return tuple(outs)

    devices = jax.devices()[:8]
    mesh = Mesh(np.asarray(devices), ("core",))
    shd = NamedSharding(mesh, PartitionSpec("core"))
    donate = tuple(range(n_params, n_params + n_outs))
    sharded = jax.jit(
        shard_map(
            _body,
            mesh=mesh,
            in_specs=(PartitionSpec("core"),) * (n_params + n_outs),
            out_specs=(PartitionSpec("core"),) * n_outs,
            check_rep=False,
        ),
        donate_argnums=donate,
        keep_unused=True,
    )
    zeros_fn = jax.jit(
        lambda: tuple(
            jnp.zeros((8 * av.shape[0],) + av.shape[1:], av.dtype) for av in out_avals
        ),
        out_shardings=(shd,) * n_outs,
    )

    cosT, sinN = _rope_tables()
    mask = _mask_table()
    ones = np.ones((128, 64), dtype=np.float32)
    statics = {
        "cosT": jax.device_put(np.tile(cosT, (8, 1)), shd),
        "sinN": jax.device_put(np.tile(sinN, (8, 1)), shd),
        "tril01": jax.device_put(np.tile(mask, (8, 1)), shd),
        "ones": jax.device_put(np.tile(ones, (8, 1)), shd),
    }
    jax.block_until_ready(list(statics.values()))

    # On-device input dedup (x shared by core pairs, W by head-groups) and
    # output pair-reduction, as separate XLA modules.
    PAIRS = [[0, 1], [2, 3], [4, 5], [6, 7]]
    QUADS = [[0, 2, 4, 6], [1, 3, 5, 7]]

    def _pre(xs, wq, wp):
        xg = jax.lax.all_gather(xs, "core", axis_index_groups=PAIRS, axis=0, tiled=True)
        wqg = jax.lax.all_gather(wq, "core", axis_index_groups=QUADS, axis=0, tiled=True)
        wpg = jax.lax.all_gather(wp, "core", axis_index_groups=QUADS, axis=0, tiled=True)
        zeros = tuple(jnp.zeros(av.shape, av.dtype) for av in out_avals)
        return (xg.T, wqg, wpg) + zeros

    pre_fn = jax.jit(
        shard_map(
            _pre,
            mesh=mesh,
            in_specs=(PartitionSpec("core"),) * 3,
            out_specs=(PartitionSpec("core"),) * (3 + n_outs),
        )
    )

    def _post(o):
        other = jax.lax.ppermute(
            o, "core", [(0, 1), (1, 0), (2, 3), (3, 2), (4, 5), (5, 4), (6, 7), (7, 6)]
        )
        s = o + other
        idx = jax.lax.axis_index("core")
        return jax.lax.dynamic_slice(s, ((idx % 2) * (T // 2), 0), (T // 2, C))

    post_fn = jax.jit(
        shard_map(
            _post,
            mesh=mesh,
            in_specs=(PartitionSpec("core"),),
            out_specs=PartitionSpec("core"),
        )
    )

    _STATE = dict(
        jax=jax,
        nc=nc,
        in_names=in_names,
        out_names=out_names,
        n_outs=n_outs,
        sharded=sharded,
        zeros_fn=zeros_fn,
        shd=shd,
        statics=statics,
        pre_fn=pre_fn,
        post_fn=post_fn,
    )
    return _STATE


def _prep_compact(x, W_attn, W_proj):
    """Compact (dedup'd) host inputs for the on-device gather pre-module.

    xs:  [8*1024, 1024] — core c gets rows of x[c//2] half c%2 (no transpose,
         no duplication; the device gathers pairs and transposes).
    wq:  [8*256, 1536]  — core c gets a quarter of wqkv_{c%2}.
    wp:  [8*128, 1024]  — core c gets a quarter of wproj_{c%2}.
    The hardware rounds fp32r matmul operands itself, so no host rounding.
    """
    xs = np.ascontiguousarray(x.reshape(8 * (T // 2), C))
    wqcat, wpcat = _prep_w(W_attn, W_proj)
    return xs, wqcat, wpcat


def _prep_w(W_attn, W_proj):
    scale = np.float32(1.0 / np.sqrt(D))
    wqkv_hg = []
    wproj_hg = []
    for hg in range(2):
        cs = slice(hg * HPC * D, (hg + 1) * HPC * D)
        wq = W_attn[:, 0 * C:][:, cs] * scale
        wk = W_attn[:, 1 * C:][:, cs]
        wv = W_attn[:, 2 * C:][:, cs]
        wqkv_hg.append(np.concatenate([wq, wk, wv], axis=1))
        wproj_hg.append(W_proj[cs, :])
    wqcat = np.empty((8 * 256, CO_QKV), dtype=np.float32)
    wpcat = np.empty((8 * 128, C), dtype=np.float32)
    for c in range(8):
        q = c // 2
        wqcat[c * 256 : (c + 1) * 256] = wqkv_hg[c % 2][q * 256 : (q + 1) * 256]
        wpcat[c * 128 : (c + 1) * 128] = wproj_hg[c % 2][q * 128 : (q + 1) * 128]
    return wqcat, wpcat


def _run_gathered(st, x, W_attn, W_proj):
    jax = st["jax"]
    # ship x first (async) so the transfer overlaps host-side W prep
    xs = np.ascontiguousarray(x.reshape(8 * (T // 2), C))
    d_xs = jax.device_put(xs, st["shd"])
    wqcat, wpcat = _prep_w(W_attn, W_proj)
    d_wq = jax.device_put(wqcat, st["shd"])
    d_wp = jax.device_put(wpcat, st["shd"])
    pre = st["pre_fn"](d_xs, d_wq, d_wp)
    dyn = {"xT": pre[0], "wqkv": pre[1], "wproj": pre[2]}
    args = [dyn[nm] if nm in dyn else st["statics"][nm] for nm in st["in_names"]]
    outs = st["sharded"](*args, *pre[3:])
    po = st["post_fn"](outs[0])
    r = np.asarray(po).reshape(B, T, C)
    return np.ascontiguousarray(r)


def _run_plain(st, x, W_attn, W_proj):
    """Fallback: duplicated per-core inputs, host-side pair sum."""
    jax = st["jax"]
    scale = np.float32(1.0 / np.sqrt(D))
    xcat = np.empty((8 * C, T), dtype=np.float32)
    for b in range(B):
        xt = np.ascontiguousarray(x[b].T)
        xcat[(2 * b) * C : (2 * b + 1) * C] = xt
        xcat[(2 * b + 1) * C : (2 * b + 2) * C] = xt
    wqkvcat = np.empty((8 * C, CO_QKV), dtype=np.float32)
    wprojcat = np.empty((8 * HPC * D, C), dtype=np.float32)
    for hg in range(2):
        cs = slice(hg * HPC * D, (hg + 1) * HPC * D)
        wq = W_attn[:, 0 * C:][:, cs] * scale
        wk = W_attn[:, 1 * C:][:, cs]
        wv = W_attn[:, 2 * C:][:, cs]
        wqkv = np.concatenate([wq, wk, wv], axis=1)
        wproj = W_proj[cs, :]
        for b in range(B):
            core = 2 * b + hg
            wqkvcat[core * C : (core + 1) * C] = wqkv
            wprojcat[core * HPC * D : (core + 1) * HPC * D] = wproj
    dyn = {"xT": xcat, "wqkv": wqkvcat, "wproj": wprojcat}
    args = [
        jax.device_put(dyn[nm], st["shd"]) if nm in dyn else st["statics"][nm]
        for nm in st["in_names"]
    ]
    zeros = st["zeros_fn"]()
    outs = st["sharded"](*args, *zeros)
    r = np.asarray(outs[0]).reshape(8, T, C)
    out = np.empty((B, T, C), dtype=np.float32)
    for b in range(B):
        np.add(r[2 * b], r[2 * b + 1], out=out[b])
    return out


def _run_rbks(x, W_attn, W_proj):
    """Last-resort fallback: the stock run_bass_kernel_spmd entry point
    (works under axon via PJRT and natively via NRT)."""
    nc = _get_nc()
    cosT, sinN = _rope_tables()
    tril = _mask_table()
    ones = np.ones((128, 64), dtype=np.float32)
    scale = np.float32(1.0 / np.sqrt(D))
    in_maps = []
    for core in range(8):
        b, hg = core // 2, core % 2
        cs = slice(hg * HPC * D, (hg + 1) * HPC * D)
        wq = W_attn[:, 0 * C:][:, cs] * scale
        wk = W_attn[:, 1 * C:][:, cs]
        wv = W_attn[:, 2 * C:][:, cs]
        in_maps.append(
            {
                "xT": np.ascontiguousarray(x[b].T),
                "wqkv": np.ascontiguousarray(np.concatenate([wq, wk, wv], axis=1)),
                "wproj": np.ascontiguousarray(W_proj[cs, :]),
                "cosT": cosT,
                "sinN": sinN,
                "tril01": tril,
                "ones": ones,
            }
        )
    res = run_bass_kernel_spmd(nc, in_maps, core_ids=list(range(8)))
    out = np.empty((B, T, C), dtype=np.float32)
    for b in range(B):
        out[b] = res.results[2 * b]["out"] + res.results[2 * b + 1]["out"]
    return out


def kernel(x, W_attn, W_proj):
    x = np.asarray(x, dtype=np.float32)
    W_attn = np.asarray(W_attn, dtype=np.float32)
    W_proj = np.asarray(W_proj, dtype=np.float32)

    try:
        st = _get_state()
        try:
            return _run_gathered(st, x, W_attn, W_proj)
        except Exception:
            return _run_plain(st, x, W_attn, W_proj)
    except Exception:
        return _run_rbks(x, W_attn, W_proj)



# revision 13
# speedup vs baseline: 1.0058x; 1.0058x over previous
"""Causal self-attention with RoPE on 8 Trainium2 NeuronCores.

Full inputs: x [4, 2048, 1024], W_attn [1024, 3072], W_proj [1024, 1024] (f32).
Sharding: core = b*2 + hg  (b in 0..3 batches, hg in 0..1 head-groups of 8 heads).
Each core computes qkv for its 8 heads, attention, and a partial output
projection (row-parallel c_proj); device pair-sum combines the two partials.

All matmuls run in bf16 (flat 1 cycle/row on the PE at any tile size, no
fp32r small-free penalty). Attention value matmul is oriented [queries, d]
with a fused ones-column denominator, normalized per-partition on DVE, then
PE-transposed for the bf16 row-parallel projection. Softmax exp is the only
Activation-engine work; phase A (QKV+RoPE) is interleaved per 512-token
chunk with attention so PE fills ACT-bound gaps.
"""

import sys

sys.path.insert(0, "/opt/trn_rl_repo")

import numpy as np
import ml_dtypes

import concourse.bass as bass  # noqa: F401
import concourse.mybir as mybir
import concourse.tile as tile
from concourse import bacc
from concourse.bass_utils import run_bass_kernel_spmd

F32 = mybir.dt.float32
BF16 = mybir.dt.bfloat16
AF = mybir.ActivationFunctionType
OP = mybir.AluOpType

B, T, C = 4, 2048, 1024
H, D = 16, 64
HPC = 8            # heads per core
CO_QKV = 3 * HPC * D   # 1536 qkv columns per core

TC = 512           # t-chunk width in phase A
N_TC = T // TC     # 4
N_KO = C // 128    # 8 contraction chunks
N_CT = 2 * HPC * D // 128   # 8 q+k column tiles (4 q, 4 k)
N_TT = T // 128    # 16 t tiles
IC = 512
N_IC = T // IC     # 4


def _rope_tables():
    """cosT f32 / sinN bf16 [128, T]: row p holds freq for d = p % 64; sinN has
    the rotate-half sign folded in (rows d<32 negative)."""
    inv_freq = (
        np.float32(1.0)
        / np.float32(10000.0) ** (np.arange(0, D, 2, dtype=np.float32) / np.float32(D))
    ).astype(np.float32)
    t = np.arange(T, dtype=np.float32)
    freqs = (t[:, None] * inv_freq[None, :]).astype(np.float32)  # [T, 32]
    emb = np.concatenate([freqs, freqs], axis=1)  # [T, 64]
    cos = np.cos(emb).astype(np.float32)
    sin = np.sin(emb).astype(np.float32)
    sinN = np.concatenate([-sin[:, :32], sin[:, 32:]], axis=1)
    cosT = np.tile(cos.T, (2, 1))   # [128, T]
    sinNT = np.tile(sinN.T, (2, 1))
    return np.ascontiguousarray(cosT), np.ascontiguousarray(sinNT)


def _mask_table():
    """tril01 [128, 128]: 1.0 if j <= c else 0.0 (key partition j, query col c)."""
    j = np.arange(128)[:, None]
    c = np.arange(128)[None, :]
    return (j <= c).astype(np.float32)


def _build():
    nc = bacc.Bacc(None, target_bir_lowering=False, debug=False)

    xT = nc.dram_tensor("xT", [C, T], BF16, kind="ExternalInput")
    wqkv = nc.dram_tensor("wqkv", [C, CO_QKV], BF16, kind="ExternalInput")
    wproj = nc.dram_tensor("wproj", [HPC * D, C], BF16, kind="ExternalInput")
    cosT_d = nc.dram_tensor("cosT", [128, T], BF16, kind="ExternalInput")
    sinN_d = nc.dram_tensor("sinN", [128, T], BF16, kind="ExternalInput")
    tril_d = nc.dram_tensor("tril01", [128, 128], BF16, kind="ExternalInput")
    ident_d = nc.dram_tensor("ident", [128, 128], BF16, kind="ExternalInput")
    out_d = nc.dram_tensor("out", [T, C], F32, kind="ExternalOutput")

    xT_r = xT.rearrange("(ko p) t -> p ko t", p=128)
    wqkv_r = wqkv.rearrange("(ko p) c -> p ko c", p=128)
    wproj_r = wproj.rearrange("(b p) c -> p b c", p=128)

    with tile.TileContext(nc) as tc:
        with (
            tc.tile_pool(name="resident", bufs=1) as res,
            tc.tile_pool(name="qkv", bufs=1) as qkv_pool,
            tc.tile_pool(name="xt", bufs=2) as xtp,
            tc.tile_pool(name="rope", bufs=6) as ropep,
            tc.tile_pool(name="exp", bufs=17) as expp,
            tc.tile_pool(name="fin", bufs=4) as finp,
            tc.tile_pool(name="ysb", bufs=6) as ysbp,
            tc.tile_pool(name="ytr", bufs=3) as ytrp,
            tc.tile_pool(name="ostage", bufs=3) as osp,
            tc.tile_pool(name="ps_a", bufs=2, space="PSUM") as psA,
            tc.tile_pool(name="ps_s", bufs=2, space="PSUM") as psS,
            tc.tile_pool(name="ps_y", bufs=2, space="PSUM") as psY,
        ):
            cos_sb = res.tile([128, T], BF16)
            sinN_sb = res.tile([128, T], BF16)
            tril_sb = res.tile([128, 128], BF16)
            ident_sb = res.tile([128, 128], BF16)
            w_sb = res.tile([128, N_KO, CO_QKV], BF16)
            wp_sb = res.tile([128, 4, C], BF16)

            # q^T / k^T: [p = d within head-pair, hp, t] bf16
            qT = qkv_pool.tile([128, HPC // 2, T], BF16)
            kT = qkv_pool.tile([128, HPC // 2, T], BF16)
            # v: [p = t%128, t//128, head, 65] with ones column at d=64
            v_sb = qkv_pool.tile([128, N_TT, HPC, D + 1], BF16)

            # ---- input DMA: first compute's inputs first ----
            xt0 = xtp.tile([128, N_KO, TC], BF16, name="xt0", tag="xt")
            for ko in range(N_KO):
                nc.sync.dma_start(xt0[:, ko], xT_r[:, ko, 0:TC])
                nc.sync.dma_start(w_sb[:, ko, 0:128], wqkv_r[:, ko, 0:128])
            for wct in range(1, 8):
                nc.sync.dma_start(
                    w_sb[:, :, wct * 128 : (wct + 1) * 128],
                    wqkv_r[:, :, wct * 128 : (wct + 1) * 128],
                )
            nc.sync.dma_start(cos_sb[:], cosT_d[:])
            nc.sync.dma_start(sinN_sb[:], sinN_d[:])
            nc.sync.dma_start(tril_sb[:], tril_d[:])
            nc.sync.dma_start(ident_sb[:], ident_d[:])
            for wct in range(8, CO_QKV // 128):
                nc.sync.dma_start(
                    w_sb[:, :, wct * 128 : (wct + 1) * 128],
                    wqkv_r[:, :, wct * 128 : (wct + 1) * 128],
                )
            nc.sync.dma_start(wp_sb[:], wproj_r[:])

            # ones columns of v (written once)
            nc.gpsimd.memset(v_sb[:, :, :, D], 1.0)

            for ic in range(N_IC):
                _phase_a_chunk(nc, ic, xtp, ropep, psA, xT_r, w_sb,
                               cos_sb, sinN_sb, qT, kT, v_sb, xt0)
                _phase_bc_chunk(nc, ic, expp, finp, ysbp, ytrp, osp,
                                psA, psS, psY, qT, kT, v_sb, tril_sb,
                                ident_sb, wp_sb, out_d)

    nc.compile()
    return nc


def _phase_a_chunk(nc, tc_i, xtp, ropep, psA, xT_r, w_sb, cos_sb, sinN_sb,
                   qT, kT, v_sb, xt0):
    ts_ = slice(tc_i * TC, (tc_i + 1) * TC)
    if tc_i == 0:
        xt_sb = xt0
    else:
        xt_sb = xtp.tile([128, N_KO, TC], BF16, name="xt", tag="xt")
        for ko in range(N_KO):
            nc.sync.dma_start(xt_sb[:, ko], xT_r[:, ko, ts_])

    # q and k column tiles (ct 0-3 -> q head-pair, 4-7 -> k)
    for ct in range(N_CT):
        psq = psA.tile([128, TC], F32, name="psq", tag="pa")
        for ko in range(N_KO):
            nc.tensor.matmul(
                psq[:],
                w_sb[:, ko, ct * 128 : (ct + 1) * 128],
                xt_sb[:, ko, :],
                start=(ko == 0),
                stop=(ko == N_KO - 1),
            )
        hp = ct % 4
        dest = (qT if ct < 4 else kT)[:, hp, ts_]
        # RoPE: dest = q * cos + shift(q) * sinN, staged through SBUF bf16
        # (only DVE/ACT can read PSUM; bf16 SBUF ops run at 2-4x on DVE)
        qraw = ropep.tile([128, TC], BF16, name="qraw", tag="qraw")
        nc.vector.tensor_copy(qraw[:], psq[:])
        rot = ropep.tile([128, TC], BF16, name="rot", tag="rot")
        for blk in range(4):
            src = (blk ^ 1) * 32
            eng = nc.vector if blk % 2 == 0 else nc.gpsimd
            eng.tensor_copy(
                rot[blk * 32 : blk * 32 + 32, :],
                qraw[src : src + 32, :],
            )
        rs = ropep.tile([128, TC], BF16, name="rs", tag="rs")
        nc.vector.tensor_tensor(rs[:], rot[:], sinN_sb[:, ts_], OP.mult)
        t2 = ropep.tile([128, TC], BF16, name="t2", tag="t2")
        nc.vector.tensor_tensor(t2[:], qraw[:], cos_sb[:, ts_], OP.mult)
        nc.vector.tensor_tensor(dest, t2[:], rs[:], OP.add)

    # v tiles: four 128-row t sub-tiles per chunk
    for sub in range(TC // 128):
        to = tc_i * (TC // 128) + sub
        psv = psA.tile([128, HPC * D], F32, name="psv", tag="pa")
        for ko in range(N_KO):
            nc.tensor.matmul(
                psv[:],
                xt_sb[:, ko, sub * 128 : sub * 128 + 128],
                w_sb[:, ko, 2 * HPC * D : 3 * HPC * D],
                start=(ko == 0),
                stop=(ko == N_KO - 1),
            )
        nc.vector.tensor_copy(
            v_sb[:, to, :, 0:D],
            psv[:].rearrange("p (h d) -> p h d", d=D),
        )


def _phase_bc_chunk(nc, ic, expp, finp, ysbp, ytrp, osp, psA, psS, psY,
                    qT, kT, v_sb, tril_sb, ident_sb, wp_sb, out_d):
    n_jt = (ic + 1) * 4
    # per-qsub output accumulators are filled across (hp, hl); staged here
    y_q = [
        ysbp.tile([128, HPC, D], BF16, name=f"yq{qs}", tag="yq")
        for qs in range(4)
    ]
    for hp in range(HPC // 2):
        # pass 1: scores + exp for every key tile (both heads of the pair)
        expts = []
        for jt in range(n_jt):
            k_diag = jt - ic * 4
            lo = max(0, k_diag * 128)
            sps = psS.tile([128, 2, TC], F32, name="sps", tag="sps")
            for hl in range(2):
                pb = hl * 64
                nc.tensor.matmul(
                    sps[:, hl, lo:],
                    kT[pb : pb + 64, hp, jt * 128 : (jt + 1) * 128],
                    qT[pb : pb + 64, hp, ic * IC + lo : (ic + 1) * IC],
                    start=True,
                    stop=True,
                )
            expT = expp.tile([128, 2, IC], BF16, name="expT", tag="expT")
            nc.scalar.activation(expT[:, :, lo:], sps[:, :, lo:], AF.Exp)
            if k_diag >= 0:
                # zero the upper-triangular part of the diagonal block
                nc.gpsimd.tensor_tensor(
                    expT[:, :, lo : lo + 128],
                    expT[:, :, lo : lo + 128],
                    tril_sb[:].rearrange("p (o c) -> p o c", o=1).broadcast_to(
                        [128, 2, 128]
                    ),
                    OP.mult,
                )
            expts.append((expT, lo))

        # pass 2: A@V oriented [queries, d+1], accumulate over key tiles
        for hl in range(2):
            for qs in range(4):
                qsg = ic * 4 + qs
                ypsum = psY.tile([128, TC], F32, name="ypsum", tag="ypsum")
                for jt in range(qsg + 1):
                    expT, _lo = expts[jt]
                    nc.tensor.matmul(
                        ypsum[:, 0 : D + 1],
                        expT[:, hl, qs * 128 : (qs + 1) * 128],
                        v_sb[:, jt, 2 * hp + hl, :],
                        start=(jt == 0),
                        stop=(jt == qsg),
                    )
                rcp = finp.tile([128, 1], F32, name="rcp", tag="rcp")
                nc.vector.reciprocal(rcp[:], ypsum[:, D : D + 1])
                nc.vector.tensor_scalar(
                    y_q[qs][:, 2 * hp + hl, :],
                    ypsum[:, 0:D],
                    rcp[:],
                    None,
                    OP.mult,
                )

    # tail: transpose y to [hd, q] and project (row-parallel partial)
    for qs in range(4):
        qt = ic * 4 + qs
        yT_sb = ytrp.tile([128, 4, 128], BF16, name="yT_sb", tag="yT_sb")
        for b in range(4):
            # each transpose owns a full psum bank (start=True zeroes 2KB)
            yT_ps = psA.tile([128, 1024], BF16, name="yT_ps", tag="pa")
            nc.tensor.transpose(
                yT_ps[:, 0:128],
                y_q[qs][:, 2 * b : 2 * b + 2, :].rearrange("p a d -> p (a d)"),
                ident_sb[:],
            )
            nc.vector.tensor_copy(yT_sb[:, b, :], yT_ps[:, 0:128])
        for cg in range(2):
            po = psA.tile([128, 512], F32, name="po", tag="pa")
            for b in range(4):
                nc.tensor.matmul(
                    po[:],
                    yT_sb[:, b, :],
                    wp_sb[:, b, cg * 512 : (cg + 1) * 512],
                    start=(b == 0),
                    stop=(b == 3),
                )
            ost = osp.tile([128, 512], F32)
            nc.scalar.copy(ost[:], po[:])
            nc.sync.dma_start(
                out_d[qt * 128 : (qt + 1) * 128, cg * 512 : (cg + 1) * 512],
                ost[:],
            )


_NC = None
_STATE = None


def _get_nc():
    global _NC
    if _NC is None:
        _NC = _build()
    return _NC


def _get_state():
    """Build the bass module once and cache a persistent jitted executor plus
    device-resident static tables (rope/mask/ones are pure functions of the
    problem shape)."""
    global _STATE
    if _STATE is not None:
        return _STATE

    import jax
    import jax.numpy as jnp
    from jax.experimental.shard_map import shard_map
    from jax.sharding import Mesh, NamedSharding, PartitionSpec

    from concourse import bass2jax

    nc = _get_nc()
    bass2jax.install_neuronx_cc_hook()
    partition_name = nc.partition_id_tensor.name if nc.partition_id_tensor else None
    in_names, out_names, out_avals = [], [], []
    for alloc in nc.m.functions[0].allocations:
        if not isinstance(alloc, mybir.MemoryLocationSet):
            continue
        name = alloc.memorylocations[0].name
        if alloc.kind == "ExternalInput":
            if name != partition_name:
                in_names.append(name)
        elif alloc.kind == "ExternalOutput":
            out_names.append(name)
            out_avals.append(
                jax.core.ShapedArray(tuple(alloc.tensor_shape), mybir.dt.np(alloc.dtype))
            )
    n_params, n_outs = len(in_names), len(out_avals)
    all_names = list(in_names) + out_names
    if partition_name:
        all_names.append(partition_name)

    def _body(*args):
        operands = list(args)
        if partition_name:
            operands.append(bass2jax.partition_id_tensor())
        outs = bass2jax._bass_exec_p.bind(
            *operands,
            out_avals=tuple(out_avals),
            in_names=tuple(all_names),
            out_names=tuple(out_names),
            lowering_input_output_aliases=(),
            sim_require_finite=True,
            sim_require_nnan=True,
            nc=nc,
        )
        return tuple(outs)

    devices = jax.devices()[:8]
    mesh = Mesh(np.asarray(devices), ("core",))
    shd = NamedSharding(mesh, PartitionSpec("core"))
    donate = tuple(range(n_params, n_params + n_outs))
    sharded = jax.jit(
        shard_map(
            _body,
            mesh=mesh,
            in_specs=(PartitionSpec("core"),) * (n_params + n_outs),
            out_specs=(PartitionSpec("core"),) * n_outs,
            check_rep=False,
        ),
        donate_argnums=donate,
        keep_unused=True,
    )
    zeros_fn = jax.jit(
        lambda: tuple(
            jnp.zeros((8 * av.shape[0],) + av.shape[1:], av.dtype) for av in out_avals
        ),
        out_shardings=(shd,) * n_outs,
    )

    cosT, sinN = _rope_tables()
    mask = _mask_table()
    ident = np.eye(128, dtype=np.float32)
    statics = {
        "cosT": jax.device_put(
            jnp.asarray(np.tile(cosT, (8, 1))).astype(jnp.bfloat16), shd
        ),
        "sinN": jax.device_put(
            jnp.asarray(np.tile(sinN, (8, 1))).astype(jnp.bfloat16), shd
        ),
        "tril01": jax.device_put(
            jnp.asarray(np.tile(mask, (8, 1))).astype(jnp.bfloat16), shd
        ),
        "ident": jax.device_put(
            jnp.asarray(np.tile(ident, (8, 1))).astype(jnp.bfloat16), shd
        ),
    }
    jax.block_until_ready(list(statics.values()))

    # On-device input dedup (x shared by core pairs, W by head-groups) and
    # output pair-reduction, as separate XLA modules.
    PAIRS = [[0, 1], [2, 3], [4, 5], [6, 7]]
    QUADS = [[0, 2, 4, 6], [1, 3, 5, 7]]

    def _pre(xs, wq, wp):
        xg = jax.lax.all_gather(xs, "core", axis_index_groups=PAIRS, axis=0, tiled=True)
        wqg = jax.lax.all_gather(wq, "core", axis_index_groups=QUADS, axis=0, tiled=True)
        wpg = jax.lax.all_gather(wp, "core", axis_index_groups=QUADS, axis=0, tiled=True)
        zeros = tuple(jnp.zeros(av.shape, av.dtype) for av in out_avals)
        return (
            xg.T.astype(jnp.bfloat16),
            wqg.astype(jnp.bfloat16),
            wpg.astype(jnp.bfloat16),
        ) + zeros

    pre_fn = jax.jit(
        shard_map(
            _pre,
            mesh=mesh,
            in_specs=(PartitionSpec("core"),) * 3,
            out_specs=(PartitionSpec("core"),) * (3 + n_outs),
        )
    )

    def _post(o):
        other = jax.lax.ppermute(
            o, "core", [(0, 1), (1, 0), (2, 3), (3, 2), (4, 5), (5, 4), (6, 7), (7, 6)]
        )
        s = o + other
        idx = jax.lax.axis_index("core")
        return jax.lax.dynamic_slice(s, ((idx % 2) * (T // 2), 0), (T // 2, C))

    post_fn = jax.jit(
        shard_map(
            _post,
            mesh=mesh,
            in_specs=(PartitionSpec("core"),),
            out_specs=PartitionSpec("core"),
        )
    )

    _STATE = dict(
        jax=jax,
        nc=nc,
        in_names=in_names,
        out_names=out_names,
        n_outs=n_outs,
        sharded=sharded,
        zeros_fn=zeros_fn,
        shd=shd,
        statics=statics,
        pre_fn=pre_fn,
        post_fn=post_fn,
    )
    return _STATE


def _prep_compact(x, W_attn, W_proj):
    """Compact (dedup'd) host inputs for the on-device gather pre-module."""
    xs = np.ascontiguousarray(x.reshape(8 * (T // 2), C))
    wqcat, wpcat = _prep_w(W_attn, W_proj)
    return xs, wqcat, wpcat


def _prep_w(W_attn, W_proj):
    scale = np.float32(1.0 / np.sqrt(D))
    wqkv_hg = []
    wproj_hg = []
    for hg in range(2):
        cs = slice(hg * HPC * D, (hg + 1) * HPC * D)
        wq = W_attn[:, 0 * C:][:, cs] * scale
        wk = W_attn[:, 1 * C:][:, cs]
        wv = W_attn[:, 2 * C:][:, cs]
        wqkv_hg.append(np.concatenate([wq, wk, wv], axis=1))
        wproj_hg.append(W_proj[cs, :])
    wqcat = np.empty((8 * 256, CO_QKV), dtype=np.float32)
    wpcat = np.empty((8 * 128, C), dtype=np.float32)
    for c in range(8):
        q = c // 2
        wqcat[c * 256 : (c + 1) * 256] = wqkv_hg[c % 2][q * 256 : (q + 1) * 256]
        wpcat[c * 128 : (c + 1) * 128] = wproj_hg[c % 2][q * 128 : (q + 1) * 128]
    return wqcat, wpcat


def _run_gathered(st, x, W_attn, W_proj):
    jax = st["jax"]
    # ship x first (async) so the transfer overlaps host-side W prep
    xs = np.ascontiguousarray(x.reshape(8 * (T // 2), C))
    d_xs = jax.device_put(xs, st["shd"])
    wqcat, wpcat = _prep_w(W_attn, W_proj)
    d_wq = jax.device_put(wqcat, st["shd"])
    d_wp = jax.device_put(wpcat, st["shd"])
    pre = st["pre_fn"](d_xs, d_wq, d_wp)
    dyn = {"xT": pre[0], "wqkv": pre[1], "wproj": pre[2]}
    args = [dyn[nm] if nm in dyn else st["statics"][nm] for nm in st["in_names"]]
    outs = st["sharded"](*args, *pre[3:])
    po = st["post_fn"](outs[0])
    r = np.asarray(po).reshape(B, T, C)
    return np.ascontiguousarray(r)


def _run_rbks(x, W_attn, W_proj):
    """Fallback: the stock run_bass_kernel_spmd entry point."""
    nc = _get_nc()
    cosT, sinN = _rope_tables()
    tril = _mask_table()
    bf = ml_dtypes.bfloat16

    def as_u16(a):
        return np.ascontiguousarray(a.astype(bf)).view(np.uint16)

    in_maps = []
    scale = np.float32(1.0 / np.sqrt(D))
    for core in range(8):
        b, hg = core // 2, core % 2
        cs = slice(hg * HPC * D, (hg + 1) * HPC * D)
        wq = W_attn[:, 0 * C:][:, cs] * scale
        wk = W_attn[:, 1 * C:][:, cs]
        wv = W_attn[:, 2 * C:][:, cs]
        in_maps.append(
            {
                "xT": as_u16(np.ascontiguousarray(x[b].T)),
                "wqkv": as_u16(np.concatenate([wq, wk, wv], axis=1)),
                "wproj": as_u16(W_proj[cs, :]),
                "cosT": as_u16(cosT),
                "sinN": as_u16(sinN),
                "tril01": as_u16(tril),
                "ident": as_u16(np.eye(128, dtype=np.float32)),
            }
        )
    res = run_bass_kernel_spmd(nc, in_maps, core_ids=list(range(8)))
    out = np.empty((B, T, C), dtype=np.float32)
    for b in range(B):
        out[b] = res.results[2 * b]["out"] + res.results[2 * b + 1]["out"]
    return out


def kernel(x, W_attn, W_proj):
    x = np.asarray(x, dtype=np.float32)
    W_attn = np.asarray(W_attn, dtype=np.float32)
    W_proj = np.asarray(W_proj, dtype=np.float32)

    try:
        st = _get_state()
        return _run_gathered(st, x, W_attn, W_proj)
    except Exception:
        return _run_rbks(x, W_attn, W_proj)


# revision 19
# speedup vs baseline: 1.1072x; 1.1008x over previous
"""Causal self-attention with RoPE on 8 Trainium2 NeuronCores.

Full inputs: x [4, 2048, 1024], W_attn [1024, 3072], W_proj [1024, 1024] (f32).
Sharding: core = b*2 + hg  (b in 0..3 batches, hg in 0..1 head-groups of 8 heads).
Each core computes qkv for its 8 heads, attention, and a partial output
projection (row-parallel c_proj); device pair-sum combines the two partials.

All matmuls run in bf16 (flat 1 cycle/row on the PE at any tile size, no
fp32r small-free penalty). Attention value matmul is oriented [queries, d]
with a fused ones-column denominator, normalized per-partition on DVE, then
PE-transposed for the bf16 row-parallel projection. Softmax exp is the only
Activation-engine work; phase A (QKV+RoPE) is interleaved per 512-token
chunk with attention so PE fills ACT-bound gaps.
"""

import sys

sys.path.insert(0, "/opt/trn_rl_repo")

import numpy as np
import ml_dtypes

import concourse.bass as bass  # noqa: F401
import concourse.mybir as mybir
import concourse.tile as tile
from concourse import bacc
from concourse.bass_utils import run_bass_kernel_spmd

F32 = mybir.dt.float32
BF16 = mybir.dt.bfloat16
AF = mybir.ActivationFunctionType
OP = mybir.AluOpType

B, T, C = 4, 2048, 1024
H, D = 16, 64
HPC = 8            # heads per core
CO_QKV = 3 * HPC * D   # 1536 qkv columns per core

TC = 512           # t-chunk width in phase A
N_TC = T // TC     # 4
N_KO = C // 128    # 8 contraction chunks
N_CT = 2 * HPC * D // 128   # 8 q+k column tiles (4 q, 4 k)
N_TT = T // 128    # 16 t tiles
IC = 512
N_IC = T // IC     # 4


def _rope_tables():
    """cosT f32 / sinN bf16 [128, T]: row p holds freq for d = p % 64; sinN has
    the rotate-half sign folded in (rows d<32 negative)."""
    inv_freq = (
        np.float32(1.0)
        / np.float32(10000.0) ** (np.arange(0, D, 2, dtype=np.float32) / np.float32(D))
    ).astype(np.float32)
    t = np.arange(T, dtype=np.float32)
    freqs = (t[:, None] * inv_freq[None, :]).astype(np.float32)  # [T, 32]
    emb = np.concatenate([freqs, freqs], axis=1)  # [T, 64]
    cos = np.cos(emb).astype(np.float32)
    sin = np.sin(emb).astype(np.float32)
    sinN = np.concatenate([-sin[:, :32], sin[:, 32:]], axis=1)
    cosT = np.tile(cos.T, (2, 1))   # [128, T]
    sinNT = np.tile(sinN.T, (2, 1))
    return np.ascontiguousarray(cosT), np.ascontiguousarray(sinNT)


def _mask_table():
    """tril01 [128, 128]: 1.0 if j <= c else 0.0 (key partition j, query col c)."""
    j = np.arange(128)[:, None]
    c = np.arange(128)[None, :]
    return (j <= c).astype(np.float32)


def _build():
    nc = bacc.Bacc(None, target_bir_lowering=False, debug=False)

    xT = nc.dram_tensor("xT", [C, T], BF16, kind="ExternalInput")
    wqkv = nc.dram_tensor("wqkv", [C, CO_QKV], BF16, kind="ExternalInput")
    wproj = nc.dram_tensor("wproj", [HPC * D, C], BF16, kind="ExternalInput")
    cosT_d = nc.dram_tensor("cosT", [128, T], BF16, kind="ExternalInput")
    sinN_d = nc.dram_tensor("sinN", [128, T], BF16, kind="ExternalInput")
    tril_d = nc.dram_tensor("tril01", [128, 128], BF16, kind="ExternalInput")
    ident_d = nc.dram_tensor("ident", [128, 128], BF16, kind="ExternalInput")
    out_d = nc.dram_tensor("out", [T, C], F32, kind="ExternalOutput")

    xT_r = xT.rearrange("(ko p) t -> p ko t", p=128)
    wqkv_r = wqkv.rearrange("(ko p) c -> p ko c", p=128)
    wproj_r = wproj.rearrange("(b p) c -> p b c", p=128)

    with tile.TileContext(nc) as tc:
        with (
            tc.tile_pool(name="resident", bufs=1) as res,
            tc.tile_pool(name="qkv", bufs=1) as qkv_pool,
            tc.tile_pool(name="xt", bufs=2) as xtp,
            tc.tile_pool(name="rope", bufs=6) as ropep,
            tc.tile_pool(name="exp", bufs=17) as expp,
            tc.tile_pool(name="fin", bufs=4) as finp,
            tc.tile_pool(name="ysb", bufs=9) as ysbp,
            tc.tile_pool(name="ytr", bufs=3) as ytrp,
            tc.tile_pool(name="ostage", bufs=3) as osp,
            tc.tile_pool(name="ps_a", bufs=2, space="PSUM") as psA,
            tc.tile_pool(name="ps_s", bufs=2, space="PSUM") as psS,
            tc.tile_pool(name="ps_y", bufs=2, space="PSUM") as psY,
        ):
            cos_sb = res.tile([128, T], BF16)
            sinN_sb = res.tile([128, T], BF16)
            tril_sb = res.tile([128, 128], BF16)
            ident_sb = res.tile([128, 128], BF16)
            w_sb = res.tile([128, N_KO, CO_QKV], BF16)
            wp_sb = res.tile([128, 4, C], BF16)

            # q^T / k^T: [p = d within head-pair, hp, t] bf16
            qT = qkv_pool.tile([128, HPC // 2, T], BF16)
            kT = qkv_pool.tile([128, HPC // 2, T], BF16)
            # v: [p = t%128, t//128, head, 65] with ones column at d=64
            v_sb = qkv_pool.tile([128, N_TT, HPC, D + 1], BF16)

            # ---- input DMA: first compute's inputs first ----
            xt0 = xtp.tile([128, N_KO, TC], BF16, name="xt0", tag="xt")
            for ko in range(N_KO):
                nc.sync.dma_start(xt0[:, ko], xT_r[:, ko, 0:TC])
                nc.sync.dma_start(w_sb[:, ko, 0:128], wqkv_r[:, ko, 0:128])
            for wct in range(1, 8):
                nc.sync.dma_start(
                    w_sb[:, :, wct * 128 : (wct + 1) * 128],
                    wqkv_r[:, :, wct * 128 : (wct + 1) * 128],
                )
            nc.sync.dma_start(cos_sb[:], cosT_d[:])
            nc.sync.dma_start(sinN_sb[:], sinN_d[:])
            nc.sync.dma_start(tril_sb[:], tril_d[:])
            nc.sync.dma_start(ident_sb[:], ident_d[:])
            for wct in range(8, CO_QKV // 128):
                nc.sync.dma_start(
                    w_sb[:, :, wct * 128 : (wct + 1) * 128],
                    wqkv_r[:, :, wct * 128 : (wct + 1) * 128],
                )
            nc.sync.dma_start(wp_sb[:], wproj_r[:])

            # ones columns of v (written once)
            nc.gpsimd.memset(v_sb[:, :, :, D], 1.0)

            # Filler queue: PE executes its queue in order, so attention
            # score passes (paced by ACT exp through the 2-deep score psum)
            # would stall PE. Interleave next-chunk QKV units and prior-chunk
            # projection tails between score tiles to keep PE fed.
            filler = []

            def emit_filler(n=1):
                for _ in range(min(n, len(filler))):
                    filler.pop(0)()

            for u in _phase_a_units(nc, 0, xtp, ropep, psA, xT_r, w_sb,
                                    cos_sb, sinN_sb, qT, kT, v_sb, xt0):
                u()
            for ic in range(N_IC):
                a_units = []
                if ic + 1 < N_IC:
                    a_units = _phase_a_units(
                        nc, ic + 1, xtp, ropep, psA, xT_r, w_sb,
                        cos_sb, sinN_sb, qT, kT, v_sb, xt0)
                filler.extend(a_units)
                tail_units = _phase_bc_chunk(
                    nc, ic, expp, finp, ysbp, ytrp, osp,
                    psA, psS, psY, qT, kT, v_sb, tril_sb,
                    ident_sb, wp_sb, out_d, emit_filler)
                # next BC chunk reads this chunk's q/k/v: flush leftovers
                emit_filler(len(filler))
                filler.extend(tail_units)
            emit_filler(len(filler))

    nc.compile()
    return nc


def _phase_a_units(nc, tc_i, xtp, ropep, psA, xT_r, w_sb, cos_sb, sinN_sb,
                   qT, kT, v_sb, xt0):
    """Return one closure per QKV unit (8 q/k column tiles + 4 v tiles)."""
    ts_ = slice(tc_i * TC, (tc_i + 1) * TC)
    if tc_i == 0:
        xt_sb = xt0
    else:
        xt_sb = xtp.tile([128, N_KO, TC], BF16, name="xt", tag="xt")
        for ko in range(N_KO):
            nc.sync.dma_start(xt_sb[:, ko], xT_r[:, ko, ts_])

    def qk_unit(ct):
        psq = psA.tile([128, TC], F32, name="psq", tag="pa")
        for ko in range(N_KO):
            nc.tensor.matmul(
                psq[:],
                w_sb[:, ko, ct * 128 : (ct + 1) * 128],
                xt_sb[:, ko, :],
                start=(ko == 0),
                stop=(ko == N_KO - 1),
            )
        hp = ct % 4
        dest = (qT if ct < 4 else kT)[:, hp, ts_]
        # RoPE: dest = q * cos + shift(q) * sinN, staged through SBUF bf16
        # (only DVE/ACT can read PSUM; bf16 SBUF ops run at 2-4x on DVE)
        qraw = ropep.tile([128, TC], BF16, name="qraw", tag="qraw")
        nc.vector.tensor_copy(qraw[:], psq[:])
        rot = ropep.tile([128, TC], BF16, name="rot", tag="rot")
        for blk in range(4):
            src = (blk ^ 1) * 32
            eng = nc.vector if blk % 2 == 0 else nc.gpsimd
            eng.tensor_copy(
                rot[blk * 32 : blk * 32 + 32, :],
                qraw[src : src + 32, :],
            )
        rs = ropep.tile([128, TC], BF16, name="rs", tag="rs")
        nc.vector.tensor_tensor(rs[:], rot[:], sinN_sb[:, ts_], OP.mult)
        t2 = ropep.tile([128, TC], BF16, name="t2", tag="t2")
        nc.vector.tensor_tensor(t2[:], qraw[:], cos_sb[:, ts_], OP.mult)
        nc.vector.tensor_tensor(dest, t2[:], rs[:], OP.add)

    def v_unit(sub):
        to = tc_i * (TC // 128) + sub
        psv = psA.tile([128, HPC * D], F32, name="psv", tag="pa")
        for ko in range(N_KO):
            nc.tensor.matmul(
                psv[:],
                xt_sb[:, ko, sub * 128 : sub * 128 + 128],
                w_sb[:, ko, 2 * HPC * D : 3 * HPC * D],
                start=(ko == 0),
                stop=(ko == N_KO - 1),
            )
        nc.vector.tensor_copy(
            v_sb[:, to, :, 0:D],
            psv[:].rearrange("p (h d) -> p h d", d=D),
        )

    units = [(lambda ct=ct: qk_unit(ct)) for ct in range(N_CT)]
    units += [(lambda sub=sub: v_unit(sub)) for sub in range(TC // 128)]
    return units


def _phase_bc_chunk(nc, ic, expp, finp, ysbp, ytrp, osp, psA, psS, psY,
                    qT, kT, v_sb, tril_sb, ident_sb, wp_sb, out_d,
                    emit_filler):
    n_jt = (ic + 1) * 4
    # per-qsub output accumulators are filled across (hp, hl); staged here
    y_q = [
        ysbp.tile([128, HPC, D], BF16, name=f"yq{qs}", tag="yq")
        for qs in range(4)
    ]
    for hp in range(HPC // 2):
        # pass 1: scores + exp for every key tile (both heads of the pair)
        expts = []
        for jt in range(n_jt):
            k_diag = jt - ic * 4
            lo = max(0, k_diag * 128)
            sps = psS.tile([128, 2, TC], F32, name="sps", tag="sps")
            for hl in range(2):
                pb = hl * 64
                nc.tensor.matmul(
                    sps[:, hl, lo:],
                    kT[pb : pb + 64, hp, jt * 128 : (jt + 1) * 128],
                    qT[pb : pb + 64, hp, ic * IC + lo : (ic + 1) * IC],
                    start=True,
                    stop=True,
                )
            expT = expp.tile([128, 2, IC], BF16, name="expT", tag="expT")
            nc.scalar.activation(expT[:, :, lo:], sps[:, :, lo:], AF.Exp)
            if k_diag >= 0:
                # zero the upper-triangular part of the diagonal block
                nc.gpsimd.tensor_tensor(
                    expT[:, :, lo : lo + 128],
                    expT[:, :, lo : lo + 128],
                    tril_sb[:].rearrange("p (o c) -> p o c", o=1).broadcast_to(
                        [128, 2, 128]
                    ),
                    OP.mult,
                )
            expts.append((expT, lo))
            if jt % 2 == 1:
                emit_filler(1)

        # pass 2: A@V oriented [queries, d+1], accumulate over key tiles
        for hl in range(2):
            for qs in range(4):
                qsg = ic * 4 + qs
                ypsum = psY.tile([128, TC], F32, name="ypsum", tag="ypsum")
                for jt in range(qsg + 1):
                    expT, _lo = expts[jt]
                    nc.tensor.matmul(
                        ypsum[:, 0 : D + 1],
                        expT[:, hl, qs * 128 : (qs + 1) * 128],
                        v_sb[:, jt, 2 * hp + hl, :],
                        start=(jt == 0),
                        stop=(jt == qsg),
                    )
                rcp = finp.tile([128, 1], F32, name="rcp", tag="rcp")
                nc.vector.reciprocal(rcp[:], ypsum[:, D : D + 1])
                nc.vector.tensor_scalar(
                    y_q[qs][:, 2 * hp + hl, :],
                    ypsum[:, 0:D],
                    rcp[:],
                    None,
                    OP.mult,
                )

    # tail: transpose y to [hd, q] and project (row-parallel partial).
    # Returned as filler closures so they slot into the next chunk's
    # ACT-paced score pass instead of serializing here.
    def tail_unit(qs):
        qt = ic * 4 + qs
        yT_sb = ytrp.tile([128, 4, 128], BF16, name="yT_sb", tag="yT_sb")
        for b in range(4):
            # each transpose owns a full psum bank (start=True zeroes 2KB)
            yT_ps = psA.tile([128, 1024], BF16, name="yT_ps", tag="pa")
            nc.tensor.transpose(
                yT_ps[:, 0:128],
                y_q[qs][:, 2 * b : 2 * b + 2, :].rearrange("p a d -> p (a d)"),
                ident_sb[:],
            )
            nc.vector.tensor_copy(yT_sb[:, b, :], yT_ps[:, 0:128])
        for cg in range(2):
            po = psA.tile([128, 512], F32, name="po", tag="pa")
            for b in range(4):
                nc.tensor.matmul(
                    po[:],
                    yT_sb[:, b, :],
                    wp_sb[:, b, cg * 512 : (cg + 1) * 512],
                    start=(b == 0),
                    stop=(b == 3),
                )
            ost = osp.tile([128, 512], F32)
            nc.scalar.copy(ost[:], po[:])
            nc.sync.dma_start(
                out_d[qt * 128 : (qt + 1) * 128, cg * 512 : (cg + 1) * 512],
                ost[:],
            )

    return [(lambda qs=qs: tail_unit(qs)) for qs in range(4)]


_NC = None
_STATE = None


def _get_nc():
    global _NC
    if _NC is None:
        _NC = _build()
    return _NC


def _get_state():
    """Build the bass module once and cache a persistent jitted executor plus
    device-resident static tables (rope/mask/ones are pure functions of the
    problem shape)."""
    global _STATE
    if _STATE is not None:
        return _STATE

    import jax
    import jax.numpy as jnp
    from jax.experimental.shard_map import shard_map
    from jax.sharding import Mesh, NamedSharding, PartitionSpec

    from concourse import bass2jax

    nc = _get_nc()
    bass2jax.install_neuronx_cc_hook()
    partition_name = nc.partition_id_tensor.name if nc.partition_id_tensor else None
    in_names, out_names, out_avals = [], [], []
    for alloc in nc.m.functions[0].allocations:
        if not isinstance(alloc, mybir.MemoryLocationSet):
            continue
        name = alloc.memorylocations[0].name
        if alloc.kind == "ExternalInput":
            if name != partition_name:
                in_names.append(name)
        elif alloc.kind == "ExternalOutput":
            out_names.append(name)
            out_avals.append(
                jax.core.ShapedArray(tuple(alloc.tensor_shape), mybir.dt.np(alloc.dtype))
            )
    n_params, n_outs = len(in_names), len(out_avals)
    all_names = list(in_names) + out_names
    if partition_name:
        all_names.append(partition_name)

    def _body(*args):
        operands = list(args)
        if partition_name:
            operands.append(bass2jax.partition_id_tensor())
        outs = bass2jax._bass_exec_p.bind(
            *operands,
            out_avals=tuple(out_avals),
            in_names=tuple(all_names),
            out_names=tuple(out_names),
            lowering_input_output_aliases=(),
            sim_require_finite=True,
            sim_require_nnan=True,
            nc=nc,
        )
        return tuple(outs)

    devices = jax.devices()[:8]
    mesh = Mesh(np.asarray(devices), ("core",))
    shd = NamedSharding(mesh, PartitionSpec("core"))
    donate = tuple(range(n_params, n_params + n_outs))
    sharded = jax.jit(
        shard_map(
            _body,
            mesh=mesh,
            in_specs=(PartitionSpec("core"),) * (n_params + n_outs),
            out_specs=(PartitionSpec("core"),) * n_outs,
            check_rep=False,
        ),
        donate_argnums=donate,
        keep_unused=True,
    )
    zeros_fn = jax.jit(
        lambda: tuple(
            jnp.zeros((8 * av.shape[0],) + av.shape[1:], av.dtype) for av in out_avals
        ),
        out_shardings=(shd,) * n_outs,
    )

    cosT, sinN = _rope_tables()
    mask = _mask_table()
    ident = np.eye(128, dtype=np.float32)
    statics = {
        "cosT": jax.device_put(
            jnp.asarray(np.tile(cosT, (8, 1))).astype(jnp.bfloat16), shd
        ),
        "sinN": jax.device_put(
            jnp.asarray(np.tile(sinN, (8, 1))).astype(jnp.bfloat16), shd
        ),
        "tril01": jax.device_put(
            jnp.asarray(np.tile(mask, (8, 1))).astype(jnp.bfloat16), shd
        ),
        "ident": jax.device_put(
            jnp.asarray(np.tile(ident, (8, 1))).astype(jnp.bfloat16), shd
        ),
    }
    jax.block_until_ready(list(statics.values()))

    # On-device input dedup (x shared by core pairs, W by head-groups) and
    # output pair-reduction, as separate XLA modules.
    PAIRS = [[0, 1], [2, 3], [4, 5], [6, 7]]
    QUADS = [[0, 2, 4, 6], [1, 3, 5, 7]]

    def _pre(xs, wq, wp):
        xg = jax.lax.all_gather(xs, "core", axis_index_groups=PAIRS, axis=0, tiled=True)
        wqg = jax.lax.all_gather(wq, "core", axis_index_groups=QUADS, axis=0, tiled=True)
        wpg = jax.lax.all_gather(wp, "core", axis_index_groups=QUADS, axis=0, tiled=True)
        zeros = tuple(jnp.zeros(av.shape, av.dtype) for av in out_avals)
        return (
            xg.T.astype(jnp.bfloat16),
            wqg.astype(jnp.bfloat16),
            wpg.astype(jnp.bfloat16),
        ) + zeros

    pre_fn = jax.jit(
        shard_map(
            _pre,
            mesh=mesh,
            in_specs=(PartitionSpec("core"),) * 3,
            out_specs=(PartitionSpec("core"),) * (3 + n_outs),
        )
    )

    def _post(o):
        other = jax.lax.ppermute(
            o, "core", [(0, 1), (1, 0), (2, 3), (3, 2), (4, 5), (5, 4), (6, 7), (7, 6)]
        )
        s = o + other
        idx = jax.lax.axis_index("core")
        return jax.lax.dynamic_slice(s, ((idx % 2) * (T // 2), 0), (T // 2, C))

    post_fn = jax.jit(
        shard_map(
            _post,
            mesh=mesh,
            in_specs=(PartitionSpec("core"),),
            out_specs=PartitionSpec("core"),
        )
    )

    _STATE = dict(
        jax=jax,
        nc=nc,
        in_names=in_names,
        out_names=out_names,
        n_outs=n_outs,
        sharded=sharded,
        zeros_fn=zeros_fn,
        shd=shd,
        statics=statics,
        pre_fn=pre_fn,
        post_fn=post_fn,
    )
    return _STATE


def _prep_compact(x, W_attn, W_proj):
    """Compact (dedup'd) host inputs for the on-device gather pre-module."""
    xs = np.ascontiguousarray(x.reshape(8 * (T // 2), C))
    wqcat, wpcat = _prep_w(W_attn, W_proj)
    return xs, wqcat, wpcat


def _prep_w(W_attn, W_proj):
    scale = np.float32(1.0 / np.sqrt(D))
    wqkv_hg = []
    wproj_hg = []
    for hg in range(2):
        cs = slice(hg * HPC * D, (hg + 1) * HPC * D)
        wq = W_attn[:, 0 * C:][:, cs] * scale
        wk = W_attn[:, 1 * C:][:, cs]
        wv = W_attn[:, 2 * C:][:, cs]
        wqkv_hg.append(np.concatenate([wq, wk, wv], axis=1))
        wproj_hg.append(W_proj[cs, :])
    wqcat = np.empty((8 * 256, CO_QKV), dtype=np.float32)
    wpcat = np.empty((8 * 128, C), dtype=np.float32)
    for c in range(8):
        q = c // 2
        wqcat[c * 256 : (c + 1) * 256] = wqkv_hg[c % 2][q * 256 : (q + 1) * 256]
        wpcat[c * 128 : (c + 1) * 128] = wproj_hg[c % 2][q * 128 : (q + 1) * 128]
    return wqcat, wpcat


def _run_gathered(st, x, W_attn, W_proj):
    jax = st["jax"]
    # ship x first (async) so the transfer overlaps host-side W prep
    xs = np.ascontiguousarray(x.reshape(8 * (T // 2), C))
    d_xs = jax.device_put(xs, st["shd"])
    wqcat, wpcat = _prep_w(W_attn, W_proj)
    d_wq = jax.device_put(wqcat, st["shd"])
    d_wp = jax.device_put(wpcat, st["shd"])
    pre = st["pre_fn"](d_xs, d_wq, d_wp)
    dyn = {"xT": pre[0], "wqkv": pre[1], "wproj": pre[2]}
    args = [dyn[nm] if nm in dyn else st["statics"][nm] for nm in st["in_names"]]
    outs = st["sharded"](*args, *pre[3:])
    po = st["post_fn"](outs[0])
    r = np.asarray(po).reshape(B, T, C)
    return np.ascontiguousarray(r)


def _run_rbks(x, W_attn, W_proj):
    """Fallback: the stock run_bass_kernel_spmd entry point."""
    nc = _get_nc()
    cosT, sinN = _rope_tables()
    tril = _mask_table()
    bf = ml_dtypes.bfloat16

    def as_u16(a):
        return np.ascontiguousarray(a.astype(bf)).view(np.uint16)

    in_maps = []
    scale = np.float32(1.0 / np.sqrt(D))
    for core in range(8):
        b, hg = core // 2, core % 2
        cs = slice(hg * HPC * D, (hg + 1) * HPC * D)
        wq = W_attn[:, 0 * C:][:, cs] * scale
        wk = W_attn[:, 1 * C:][:, cs]
        wv = W_attn[:, 2 * C:][:, cs]
        in_maps.append(
            {
                "xT": as_u16(np.ascontiguousarray(x[b].T)),
                "wqkv": as_u16(np.concatenate([wq, wk, wv], axis=1)),
                "wproj": as_u16(W_proj[cs, :]),
                "cosT": as_u16(cosT),
                "sinN": as_u16(sinN),
                "tril01": as_u16(tril),
                "ident": as_u16(np.eye(128, dtype=np.float32)),
            }
        )
    res = run_bass_kernel_spmd(nc, in_maps, core_ids=list(range(8)))
    out = np.empty((B, T, C), dtype=np.float32)
    for b in range(B):
        out[b] = res.results[2 * b]["out"] + res.results[2 * b + 1]["out"]
    return out


def kernel(x, W_attn, W_proj):
    x = np.asarray(x, dtype=np.float32)
    W_attn = np.asarray(W_attn, dtype=np.float32)
    W_proj = np.asarray(W_proj, dtype=np.float32)

    try:
        st = _get_state()
        return _run_gathered(st, x, W_attn, W_proj)
    except Exception:
        return _run_rbks(x, W_attn, W_proj)


# revision 20
# speedup vs baseline: 1.1097x; 1.0022x over previous
"""Causal self-attention with RoPE on 8 Trainium2 NeuronCores.

Full inputs: x [4, 2048, 1024], W_attn [1024, 3072], W_proj [1024, 1024] (f32).
Sharding: core = b*2 + hg  (b in 0..3 batches, hg in 0..1 head-groups of 8 heads).
Each core computes qkv for its 8 heads, attention, and a partial output
projection (row-parallel c_proj); device pair-sum combines the two partials.

All matmuls run in bf16 (flat 1 cycle/row on the PE at any tile size, no
fp32r small-free penalty). Attention value matmul is oriented [queries, d]
with a fused ones-column denominator, normalized per-partition on DVE, then
PE-transposed for the bf16 row-parallel projection. Softmax exp is the only
Activation-engine work; phase A (QKV+RoPE) is interleaved per 512-token
chunk with attention so PE fills ACT-bound gaps.
"""

import sys

sys.path.insert(0, "/opt/trn_rl_repo")

import numpy as np
import ml_dtypes

import concourse.bass as bass  # noqa: F401
import concourse.mybir as mybir
import concourse.tile as tile
from concourse import bacc
from concourse.bass_utils import run_bass_kernel_spmd

F32 = mybir.dt.float32
BF16 = mybir.dt.bfloat16
AF = mybir.ActivationFunctionType
OP = mybir.AluOpType

B, T, C = 4, 2048, 1024
H, D = 16, 64
HPC = 8            # heads per core
CO_QKV = 3 * HPC * D   # 1536 qkv columns per core

TC = 512           # t-chunk width in phase A
N_TC = T // TC     # 4
N_KO = C // 128    # 8 contraction chunks
N_CT = 2 * HPC * D // 128   # 8 q+k column tiles (4 q, 4 k)
N_TT = T // 128    # 16 t tiles
IC = 512
N_IC = T // IC     # 4


def _rope_tables():
    """cosT f32 / sinN bf16 [128, T]: row p holds freq for d = p % 64; sinN has
    the rotate-half sign folded in (rows d<32 negative)."""
    inv_freq = (
        np.float32(1.0)
        / np.float32(10000.0) ** (np.arange(0, D, 2, dtype=np.float32) / np.float32(D))
    ).astype(np.float32)
    t = np.arange(T, dtype=np.float32)
    freqs = (t[:, None] * inv_freq[None, :]).astype(np.float32)  # [T, 32]
    emb = np.concatenate([freqs, freqs], axis=1)  # [T, 64]
    cos = np.cos(emb).astype(np.float32)
    sin = np.sin(emb).astype(np.float32)
    sinN = np.concatenate([-sin[:, :32], sin[:, 32:]], axis=1)
    cosT = np.tile(cos.T, (2, 1))   # [128, T]
    sinNT = np.tile(sinN.T, (2, 1))
    return np.ascontiguousarray(cosT), np.ascontiguousarray(sinNT)


def _mask_table():
    """tril01 [128, 128]: 1.0 if j <= c else 0.0 (key partition j, query col c)."""
    j = np.arange(128)[:, None]
    c = np.arange(128)[None, :]
    return (j <= c).astype(np.float32)


def _build():
    nc = bacc.Bacc(None, target_bir_lowering=False, debug=False)

    xT = nc.dram_tensor("xT", [C, T], BF16, kind="ExternalInput")
    wqkv = nc.dram_tensor("wqkv", [C, CO_QKV], BF16, kind="ExternalInput")
    wproj = nc.dram_tensor("wproj", [HPC * D, C], BF16, kind="ExternalInput")
    cosT_d = nc.dram_tensor("cosT", [128, T], BF16, kind="ExternalInput")
    sinN_d = nc.dram_tensor("sinN", [128, T], BF16, kind="ExternalInput")
    tril_d = nc.dram_tensor("tril01", [128, 128], BF16, kind="ExternalInput")
    ident_d = nc.dram_tensor("ident", [128, 128], BF16, kind="ExternalInput")
    out_d = nc.dram_tensor("out", [T, C], F32, kind="ExternalOutput")

    xT_r = xT.rearrange("(ko p) t -> p ko t", p=128)
    wqkv_r = wqkv.rearrange("(ko p) c -> p ko c", p=128)
    wproj_r = wproj.rearrange("(b p) c -> p b c", p=128)

    with tile.TileContext(nc) as tc:
        with (
            tc.tile_pool(name="resident", bufs=1) as res,
            tc.tile_pool(name="qkv", bufs=1) as qkv_pool,
            tc.tile_pool(name="xt", bufs=2) as xtp,
            tc.tile_pool(name="rope", bufs=6) as ropep,
            tc.tile_pool(name="exp", bufs=17) as expp,
            tc.tile_pool(name="fin", bufs=4) as finp,
            tc.tile_pool(name="ysb", bufs=9) as ysbp,
            tc.tile_pool(name="ytr", bufs=3) as ytrp,
            tc.tile_pool(name="ostage", bufs=3) as osp,
            tc.tile_pool(name="ps_a", bufs=2, space="PSUM") as psA,
            tc.tile_pool(name="ps_s", bufs=2, space="PSUM") as psS,
            tc.tile_pool(name="ps_y", bufs=2, space="PSUM") as psY,
        ):
            cos_sb = res.tile([128, T], BF16)
            sinN_sb = res.tile([128, T], BF16)
            tril_sb = res.tile([128, 128], BF16)
            ident_sb = res.tile([128, 128], BF16)
            w_sb = res.tile([128, N_KO, CO_QKV], BF16)
            wp_sb = res.tile([128, 4, C], BF16)

            # q^T / k^T: [p = d within head-pair, hp, t] bf16
            qT = qkv_pool.tile([128, HPC // 2, T], BF16)
            kT = qkv_pool.tile([128, HPC // 2, T], BF16)
            # v: [p = t%128, t//128, head, 65] with ones column at d=64
            v_sb = qkv_pool.tile([128, N_TT, HPC, D + 1], BF16)

            # ---- input DMA: first compute's inputs first ----
            xt0 = xtp.tile([128, N_KO, TC], BF16, name="xt0", tag="xt")
            for ko in range(N_KO):
                nc.sync.dma_start(xt0[:, ko], xT_r[:, ko, 0:TC])
                nc.sync.dma_start(w_sb[:, ko, 0:128], wqkv_r[:, ko, 0:128])
            for wct in range(1, 8):
                nc.sync.dma_start(
                    w_sb[:, :, wct * 128 : (wct + 1) * 128],
                    wqkv_r[:, :, wct * 128 : (wct + 1) * 128],
                )
            nc.sync.dma_start(cos_sb[:], cosT_d[:])
            nc.sync.dma_start(sinN_sb[:], sinN_d[:])
            nc.sync.dma_start(tril_sb[:], tril_d[:])
            nc.sync.dma_start(ident_sb[:], ident_d[:])
            for wct in range(8, CO_QKV // 128):
                nc.sync.dma_start(
                    w_sb[:, :, wct * 128 : (wct + 1) * 128],
                    wqkv_r[:, :, wct * 128 : (wct + 1) * 128],
                )
            nc.sync.dma_start(wp_sb[:], wproj_r[:])

            # ones columns of v (written once)
            nc.gpsimd.memset(v_sb[:, :, :, D], 1.0)

            # Filler queue: PE executes its queue in order, so attention
            # score passes (paced by ACT exp through the 2-deep score psum)
            # would stall PE. Interleave next-chunk QKV units and prior-chunk
            # projection tails between score tiles to keep PE fed.
            filler = []

            def emit_filler(n=1):
                for _ in range(min(n, len(filler))):
                    filler.pop(0)()

            for u in _phase_a_units(nc, 0, xtp, ropep, psA, xT_r, w_sb,
                                    cos_sb, sinN_sb, qT, kT, v_sb, xt0):
                u()
            for ic in range(N_IC):
                a_units = []
                if ic + 1 < N_IC:
                    a_units = _phase_a_units(
                        nc, ic + 1, xtp, ropep, psA, xT_r, w_sb,
                        cos_sb, sinN_sb, qT, kT, v_sb, xt0)
                filler.extend(a_units)
                tail_units = _phase_bc_chunk(
                    nc, ic, expp, finp, ysbp, ytrp, osp,
                    psA, psS, psY, qT, kT, v_sb, tril_sb,
                    ident_sb, wp_sb, out_d, emit_filler)
                # next BC chunk reads this chunk's q/k/v: flush leftovers
                emit_filler(len(filler))
                filler.extend(tail_units)
            emit_filler(len(filler))

    nc.compile()
    return nc


def _phase_a_units(nc, tc_i, xtp, ropep, psA, xT_r, w_sb, cos_sb, sinN_sb,
                   qT, kT, v_sb, xt0):
    """Return one closure per QKV unit (8 q/k column tiles + 4 v tiles)."""
    ts_ = slice(tc_i * TC, (tc_i + 1) * TC)
    if tc_i == 0:
        xt_sb = xt0
    else:
        xt_sb = xtp.tile([128, N_KO, TC], BF16, name="xt", tag="xt")
        for ko in range(N_KO):
            nc.sync.dma_start(xt_sb[:, ko], xT_r[:, ko, ts_])

    def qk_unit(ct):
        psq = psA.tile([128, TC], F32, name="psq", tag="pa")
        for ko in range(N_KO):
            nc.tensor.matmul(
                psq[:],
                w_sb[:, ko, ct * 128 : (ct + 1) * 128],
                xt_sb[:, ko, :],
                start=(ko == 0),
                stop=(ko == N_KO - 1),
            )
        hp = ct % 4
        dest = (qT if ct < 4 else kT)[:, hp, ts_]
        # RoPE: dest = q * cos + shift(q) * sinN, staged through SBUF bf16
        # (only DVE/ACT can read PSUM; bf16 SBUF ops run at 2-4x on DVE)
        qraw = ropep.tile([128, TC], BF16, name="qraw", tag="qraw")
        nc.vector.tensor_copy(qraw[:], psq[:])
        rot = ropep.tile([128, TC], BF16, name="rot", tag="rot")
        for blk in range(4):
            src = (blk ^ 1) * 32
            eng = nc.vector if blk % 2 == 0 else nc.gpsimd
            eng.tensor_copy(
                rot[blk * 32 : blk * 32 + 32, :],
                qraw[src : src + 32, :],
            )
        rs = ropep.tile([128, TC], BF16, name="rs", tag="rs")
        nc.vector.tensor_tensor(rs[:], rot[:], sinN_sb[:, ts_], OP.mult)
        t2 = ropep.tile([128, TC], BF16, name="t2", tag="t2")
        nc.vector.tensor_tensor(t2[:], qraw[:], cos_sb[:, ts_], OP.mult)
        nc.vector.tensor_tensor(dest, t2[:], rs[:], OP.add)

    def v_unit(sub):
        to = tc_i * (TC // 128) + sub
        psv = psA.tile([128, HPC * D], F32, name="psv", tag="pa")
        for ko in range(N_KO):
            nc.tensor.matmul(
                psv[:],
                xt_sb[:, ko, sub * 128 : sub * 128 + 128],
                w_sb[:, ko, 2 * HPC * D : 3 * HPC * D],
                start=(ko == 0),
                stop=(ko == N_KO - 1),
            )
        nc.vector.tensor_copy(
            v_sb[:, to, :, 0:D],
            psv[:].rearrange("p (h d) -> p h d", d=D),
        )

    units = [(lambda ct=ct: qk_unit(ct)) for ct in range(N_CT)]
    units += [(lambda sub=sub: v_unit(sub)) for sub in range(TC // 128)]
    return units


def _phase_bc_chunk(nc, ic, expp, finp, ysbp, ytrp, osp, psA, psS, psY,
                    qT, kT, v_sb, tril_sb, ident_sb, wp_sb, out_d,
                    emit_filler):
    n_jt = (ic + 1) * 4
    # per-qsub output accumulators are filled across (hp, hl); staged here
    y_q = [
        ysbp.tile([128, HPC, D], BF16, name=f"yq{qs}", tag="yq")
        for qs in range(4)
    ]
    for hp in range(HPC // 2):
        # pass 1: scores + exp for every key tile (both heads of the pair)
        expts = []
        for jt in range(n_jt):
            k_diag = jt - ic * 4
            lo = max(0, k_diag * 128)
            sps = psS.tile([128, 2, TC], F32, name="sps", tag="sps")
            for hl in range(2):
                pb = hl * 64
                nc.tensor.matmul(
                    sps[:, hl, lo:],
                    kT[pb : pb + 64, hp, jt * 128 : (jt + 1) * 128],
                    qT[pb : pb + 64, hp, ic * IC + lo : (ic + 1) * IC],
                    start=True,
                    stop=True,
                )
            expT = expp.tile([128, 2, IC], BF16, name="expT", tag="expT")
            nc.scalar.activation(expT[:, :, lo:], sps[:, :, lo:], AF.Exp)
            if k_diag >= 0:
                # zero the upper-triangular part of the diagonal block
                nc.gpsimd.tensor_tensor(
                    expT[:, :, lo : lo + 128],
                    expT[:, :, lo : lo + 128],
                    tril_sb[:].rearrange("p (o c) -> p o c", o=1).broadcast_to(
                        [128, 2, 128]
                    ),
                    OP.mult,
                )
            expts.append((expT, lo))
            if jt % 2 == 1:
                emit_filler(1)

        # pass 2: A@V oriented [queries, d+1], accumulate over key tiles
        for hl in range(2):
            for qs in range(4):
                qsg = ic * 4 + qs
                ypsum = psY.tile([128, TC], F32, name="ypsum", tag="ypsum")
                for jt in range(qsg + 1):
                    expT, _lo = expts[jt]
                    nc.tensor.matmul(
                        ypsum[:, 0 : D + 1],
                        expT[:, hl, qs * 128 : (qs + 1) * 128],
                        v_sb[:, jt, 2 * hp + hl, :],
                        start=(jt == 0),
                        stop=(jt == qsg),
                    )
                rcp = finp.tile([128, 1], F32, name="rcp", tag="rcp")
                nc.vector.reciprocal(rcp[:], ypsum[:, D : D + 1])
                nc.vector.tensor_scalar(
                    y_q[qs][:, 2 * hp + hl, :],
                    ypsum[:, 0:D],
                    rcp[:],
                    None,
                    OP.mult,
                )

    # tail: transpose y to [hd, q] and project (row-parallel partial).
    # Returned as filler closures so they slot into the next chunk's
    # ACT-paced score pass instead of serializing here.
    def tail_unit(qs):
        qt = ic * 4 + qs
        yT_sb = ytrp.tile([128, 4, 128], BF16, name="yT_sb", tag="yT_sb")
        for b in range(4):
            # each transpose owns a full psum bank (start=True zeroes 2KB)
            yT_ps = psA.tile([128, 1024], BF16, name="yT_ps", tag="pa")
            nc.tensor.transpose(
                yT_ps[:, 0:128],
                y_q[qs][:, 2 * b : 2 * b + 2, :].rearrange("p a d -> p (a d)"),
                ident_sb[:],
            )
            nc.vector.tensor_copy(yT_sb[:, b, :], yT_ps[:, 0:128])
        for cg in range(2):
            po = psA.tile([128, 512], F32, name="po", tag="pa")
            for b in range(4):
                nc.tensor.matmul(
                    po[:],
                    yT_sb[:, b, :],
                    wp_sb[:, b, cg * 512 : (cg + 1) * 512],
                    start=(b == 0),
                    stop=(b == 3),
                )
            ost = osp.tile([128, 512], F32)
            nc.vector.tensor_copy(ost[:], po[:])
            nc.sync.dma_start(
                out_d[qt * 128 : (qt + 1) * 128, cg * 512 : (cg + 1) * 512],
                ost[:],
            )

    return [(lambda qs=qs: tail_unit(qs)) for qs in range(4)]


_NC = None
_STATE = None


def _get_nc():
    global _NC
    if _NC is None:
        _NC = _build()
    return _NC


def _get_state():
    """Build the bass module once and cache a persistent jitted executor plus
    device-resident static tables (rope/mask/ones are pure functions of the
    problem shape)."""
    global _STATE
    if _STATE is not None:
        return _STATE

    import jax
    import jax.numpy as jnp
    from jax.experimental.shard_map import shard_map
    from jax.sharding import Mesh, NamedSharding, PartitionSpec

    from concourse import bass2jax

    nc = _get_nc()
    bass2jax.install_neuronx_cc_hook()
    partition_name = nc.partition_id_tensor.name if nc.partition_id_tensor else None
    in_names, out_names, out_avals = [], [], []
    for alloc in nc.m.functions[0].allocations:
        if not isinstance(alloc, mybir.MemoryLocationSet):
            continue
        name = alloc.memorylocations[0].name
        if alloc.kind == "ExternalInput":
            if name != partition_name:
                in_names.append(name)
        elif alloc.kind == "ExternalOutput":
            out_names.append(name)
            out_avals.append(
                jax.core.ShapedArray(tuple(alloc.tensor_shape), mybir.dt.np(alloc.dtype))
            )
    n_params, n_outs = len(in_names), len(out_avals)
    all_names = list(in_names) + out_names
    if partition_name:
        all_names.append(partition_name)

    def _body(*args):
        operands = list(args)
        if partition_name:
            operands.append(bass2jax.partition_id_tensor())
        outs = bass2jax._bass_exec_p.bind(
            *operands,
            out_avals=tuple(out_avals),
            in_names=tuple(all_names),
            out_names=tuple(out_names),
            lowering_input_output_aliases=(),
            sim_require_finite=True,
            sim_require_nnan=True,
            nc=nc,
        )
        return tuple(outs)

    devices = jax.devices()[:8]
    mesh = Mesh(np.asarray(devices), ("core",))
    shd = NamedSharding(mesh, PartitionSpec("core"))
    donate = tuple(range(n_params, n_params + n_outs))
    sharded = jax.jit(
        shard_map(
            _body,
            mesh=mesh,
            in_specs=(PartitionSpec("core"),) * (n_params + n_outs),
            out_specs=(PartitionSpec("core"),) * n_outs,
            check_rep=False,
        ),
        donate_argnums=donate,
        keep_unused=True,
    )
    zeros_fn = jax.jit(
        lambda: tuple(
            jnp.zeros((8 * av.shape[0],) + av.shape[1:], av.dtype) for av in out_avals
        ),
        out_shardings=(shd,) * n_outs,
    )

    cosT, sinN = _rope_tables()
    mask = _mask_table()
    ident = np.eye(128, dtype=np.float32)
    statics = {
        "cosT": jax.device_put(
            jnp.asarray(np.tile(cosT, (8, 1))).astype(jnp.bfloat16), shd
        ),
        "sinN": jax.device_put(
            jnp.asarray(np.tile(sinN, (8, 1))).astype(jnp.bfloat16), shd
        ),
        "tril01": jax.device_put(
            jnp.asarray(np.tile(mask, (8, 1))).astype(jnp.bfloat16), shd
        ),
        "ident": jax.device_put(
            jnp.asarray(np.tile(ident, (8, 1))).astype(jnp.bfloat16), shd
        ),
    }
    jax.block_until_ready(list(statics.values()))

    # On-device input dedup (x shared by core pairs, W by head-groups) and
    # output pair-reduction, as separate XLA modules.
    PAIRS = [[0, 1], [2, 3], [4, 5], [6, 7]]
    QUADS = [[0, 2, 4, 6], [1, 3, 5, 7]]

    def _pre(xs, wq, wp):
        xg = jax.lax.all_gather(xs, "core", axis_index_groups=PAIRS, axis=0, tiled=True)
        wqg = jax.lax.all_gather(wq, "core", axis_index_groups=QUADS, axis=0, tiled=True)
        wpg = jax.lax.all_gather(wp, "core", axis_index_groups=QUADS, axis=0, tiled=True)
        zeros = tuple(jnp.zeros(av.shape, av.dtype) for av in out_avals)
        return (
            xg.T.astype(jnp.bfloat16),
            wqg.astype(jnp.bfloat16),
            wpg.astype(jnp.bfloat16),
        ) + zeros

    pre_fn = jax.jit(
        shard_map(
            _pre,
            mesh=mesh,
            in_specs=(PartitionSpec("core"),) * 3,
            out_specs=(PartitionSpec("core"),) * (3 + n_outs),
        )
    )

    def _post(o):
        other = jax.lax.ppermute(
            o, "core", [(0, 1), (1, 0), (2, 3), (3, 2), (4, 5), (5, 4), (6, 7), (7, 6)]
        )
        s = o + other
        idx = jax.lax.axis_index("core")
        return jax.lax.dynamic_slice(s, ((idx % 2) * (T // 2), 0), (T // 2, C))

    post_fn = jax.jit(
        shard_map(
            _post,
            mesh=mesh,
            in_specs=(PartitionSpec("core"),),
            out_specs=PartitionSpec("core"),
        )
    )

    _STATE = dict(
        jax=jax,
        nc=nc,
        in_names=in_names,
        out_names=out_names,
        n_outs=n_outs,
        sharded=sharded,
        zeros_fn=zeros_fn,
        shd=shd,
        statics=statics,
        pre_fn=pre_fn,
        post_fn=post_fn,
    )
    return _STATE


def _prep_compact(x, W_attn, W_proj):
    """Compact (dedup'd) host inputs for the on-device gather pre-module."""
    xs = np.ascontiguousarray(x.reshape(8 * (T // 2), C))
    wqcat, wpcat = _prep_w(W_attn, W_proj)
    return xs, wqcat, wpcat


def _prep_w(W_attn, W_proj):
    scale = np.float32(1.0 / np.sqrt(D))
    wqkv_hg = []
    wproj_hg = []
    for hg in range(2):
        cs = slice(hg * HPC * D, (hg + 1) * HPC * D)
        wq = W_attn[:, 0 * C:][:, cs] * scale
        wk = W_attn[:, 1 * C:][:, cs]
        wv = W_attn[:, 2 * C:][:, cs]
        wqkv_hg.append(np.concatenate([wq, wk, wv], axis=1))
        wproj_hg.append(W_proj[cs, :])
    wqcat = np.empty((8 * 256, CO_QKV), dtype=np.float32)
    wpcat = np.empty((8 * 128, C), dtype=np.float32)
    for c in range(8):
        q = c // 2
        wqcat[c * 256 : (c + 1) * 256] = wqkv_hg[c % 2][q * 256 : (q + 1) * 256]
        wpcat[c * 128 : (c + 1) * 128] = wproj_hg[c % 2][q * 128 : (q + 1) * 128]
    return wqcat, wpcat


def _run_gathered(st, x, W_attn, W_proj):
    jax = st["jax"]
    # ship x first (async) so the transfer overlaps host-side W prep
    xs = np.ascontiguousarray(x.reshape(8 * (T // 2), C))
    d_xs = jax.device_put(xs, st["shd"])
    wqcat, wpcat = _prep_w(W_attn, W_proj)
    d_wq = jax.device_put(wqcat, st["shd"])
    d_wp = jax.device_put(wpcat, st["shd"])
    pre = st["pre_fn"](d_xs, d_wq, d_wp)
    dyn = {"xT": pre[0], "wqkv": pre[1], "wproj": pre[2]}
    args = [dyn[nm] if nm in dyn else st["statics"][nm] for nm in st["in_names"]]
    outs = st["sharded"](*args, *pre[3:])
    po = st["post_fn"](outs[0])
    r = np.asarray(po).reshape(B, T, C)
    return np.ascontiguousarray(r)


def _run_rbks(x, W_attn, W_proj):
    """Fallback: the stock run_bass_kernel_spmd entry point."""
    nc = _get_nc()
    cosT, sinN = _rope_tables()
    tril = _mask_table()
    bf = ml_dtypes.bfloat16

    def as_u16(a):
        return np.ascontiguousarray(a.astype(bf)).view(np.uint16)

    in_maps = []
    scale = np.float32(1.0 / np.sqrt(D))
    for core in range(8):
        b, hg = core // 2, core % 2
        cs = slice(hg * HPC * D, (hg + 1) * HPC * D)
        wq = W_attn[:, 0 * C:][:, cs] * scale
        wk = W_attn[:, 1 * C:][:, cs]
        wv = W_attn[:, 2 * C:][:, cs]
        in_maps.append(
            {
                "xT": as_u16(np.ascontiguousarray(x[b].T)),
                "wqkv": as_u16(np.concatenate([wq, wk, wv], axis=1)),
                "wproj": as_u16(W_proj[cs, :]),
                "cosT": as_u16(cosT),
                "sinN": as_u16(sinN),
                "tril01": as_u16(tril),
                "ident": as_u16(np.eye(128, dtype=np.float32)),
            }
        )
    res = run_bass_kernel_spmd(nc, in_maps, core_ids=list(range(8)))
    out = np.empty((B, T, C), dtype=np.float32)
    for b in range(B):
        out[b] = res.results[2 * b]["out"] + res.results[2 * b + 1]["out"]
    return out


def kernel(x, W_attn, W_proj):
    x = np.asarray(x, dtype=np.float32)
    W_attn = np.asarray(W_attn, dtype=np.float32)
    W_proj = np.asarray(W_proj, dtype=np.float32)

    try:
        st = _get_state()
        return _run_gathered(st, x, W_attn, W_proj)
    except Exception:
        return _run_rbks(x, W_attn, W_proj)


# revision 25
# speedup vs baseline: 1.1120x; 1.0021x over previous
"""Causal self-attention with RoPE on 8 Trainium2 NeuronCores.

Full inputs: x [4, 2048, 1024], W_attn [1024, 3072], W_proj [1024, 1024] (f32).
Sharding: core = b*2 + hg  (b in 0..3 batches, hg in 0..1 head-groups of 8 heads).
Each core computes qkv for its 8 heads, attention, and a partial output
projection (row-parallel c_proj); device pair-sum combines the two partials.

All matmuls run in bf16 (flat 1 cycle/row on the PE at any tile size, no
fp32r small-free penalty). Attention value matmul is oriented [queries, d]
with a fused ones-column denominator, normalized per-partition on DVE, then
PE-transposed for the bf16 row-parallel projection. Softmax exp is the only
Activation-engine work; phase A (QKV+RoPE) is interleaved per 512-token
chunk with attention so PE fills ACT-bound gaps.
"""

import sys

sys.path.insert(0, "/opt/trn_rl_repo")

import numpy as np
import ml_dtypes

import concourse.bass as bass  # noqa: F401
import concourse.mybir as mybir
import concourse.tile as tile
from concourse import bacc
from concourse.bass_utils import run_bass_kernel_spmd

F32 = mybir.dt.float32
BF16 = mybir.dt.bfloat16
AF = mybir.ActivationFunctionType
OP = mybir.AluOpType

B, T, C = 4, 2048, 1024
H, D = 16, 64
HPC = 8            # heads per core
CO_QKV = 3 * HPC * D   # 1536 qkv columns per core

TC = 512           # t-chunk width in phase A
N_TC = T // TC     # 4
N_KO = C // 128    # 8 contraction chunks
N_CT = 2 * HPC * D // 128   # 8 q+k column tiles (4 q, 4 k)
N_TT = T // 128    # 16 t tiles
IC = 512
N_IC = T // IC     # 4


def _rope_tables():
    """cosT f32 / sinN bf16 [128, T]: row p holds freq for d = p % 64; sinN has
    the rotate-half sign folded in (rows d<32 negative)."""
    inv_freq = (
        np.float32(1.0)
        / np.float32(10000.0) ** (np.arange(0, D, 2, dtype=np.float32) / np.float32(D))
    ).astype(np.float32)
    t = np.arange(T, dtype=np.float32)
    freqs = (t[:, None] * inv_freq[None, :]).astype(np.float32)  # [T, 32]
    emb = np.concatenate([freqs, freqs], axis=1)  # [T, 64]
    cos = np.cos(emb).astype(np.float32)
    sin = np.sin(emb).astype(np.float32)
    sinN = np.concatenate([-sin[:, :32], sin[:, 32:]], axis=1)
    cosT = np.tile(cos.T, (2, 1))   # [128, T]
    sinNT = np.tile(sinN.T, (2, 1))
    return np.ascontiguousarray(cosT), np.ascontiguousarray(sinNT)


def _mask_table():
    """tril01 [128, 128]: 1.0 if j <= c else 0.0 (key partition j, query col c)."""
    j = np.arange(128)[:, None]
    c = np.arange(128)[None, :]
    return (j <= c).astype(np.float32)


def _build():
    nc = bacc.Bacc(None, target_bir_lowering=False, debug=False)

    xT = nc.dram_tensor("xT", [C, T], BF16, kind="ExternalInput")
    wqkv = nc.dram_tensor("wqkv", [C, CO_QKV], BF16, kind="ExternalInput")
    wproj = nc.dram_tensor("wproj", [HPC * D, C], BF16, kind="ExternalInput")
    cosT_d = nc.dram_tensor("cosT", [128, T], BF16, kind="ExternalInput")
    sinN_d = nc.dram_tensor("sinN", [128, T], BF16, kind="ExternalInput")
    tril_d = nc.dram_tensor("tril01", [128, 128], BF16, kind="ExternalInput")
    ident_d = nc.dram_tensor("ident", [128, 128], BF16, kind="ExternalInput")
    out_d = nc.dram_tensor("out", [T, C], F32, kind="ExternalOutput")

    xT_r = xT.rearrange("(ko p) t -> p ko t", p=128)
    wqkv_r = wqkv.rearrange("(ko p) c -> p ko c", p=128)
    wproj_r = wproj.rearrange("(b p) c -> p b c", p=128)

    with tile.TileContext(nc) as tc:
        with (
            tc.tile_pool(name="resident", bufs=1) as res,
            tc.tile_pool(name="qkv", bufs=1) as qkv_pool,
            tc.tile_pool(name="xt", bufs=2) as xtp,
            tc.tile_pool(name="rope", bufs=5) as ropep,
            tc.tile_pool(name="exp", bufs=33) as expp,
            tc.tile_pool(name="fin", bufs=4) as finp,
            tc.tile_pool(name="ysb", bufs=9) as ysbp,
            tc.tile_pool(name="ytr", bufs=2) as ytrp,
            tc.tile_pool(name="ostage", bufs=2) as osp,
            tc.tile_pool(name="ps_a", bufs=2, space="PSUM") as psA,
            tc.tile_pool(name="ps_s", bufs=2, space="PSUM") as psS,
            tc.tile_pool(name="ps_y", bufs=2, space="PSUM") as psY,
        ):
            cos_sb = res.tile([128, T], BF16)
            sinN_sb = res.tile([128, T], BF16)
            tril_sb = res.tile([128, 128], BF16)
            ident_sb = res.tile([128, 128], BF16)
            w_sb = res.tile([128, N_KO, CO_QKV], BF16)
            wp_sb = res.tile([128, 4, C], BF16)

            # q^T / k^T: [p = d within head-pair, hp, t] bf16
            qT = qkv_pool.tile([128, HPC // 2, T], BF16)
            kT = qkv_pool.tile([128, HPC // 2, T], BF16)
            # v: [p = t%128, t//128, head, 65] with ones column at d=64
            v_sb = qkv_pool.tile([128, N_TT, HPC, D + 1], BF16)

            # ---- input DMA: first compute's inputs first ----
            xt0 = xtp.tile([128, N_KO, TC], BF16, name="xt0", tag="xt")
            for ko in range(N_KO):
                nc.sync.dma_start(xt0[:, ko], xT_r[:, ko, 0:TC])
                nc.sync.dma_start(w_sb[:, ko, 0:128], wqkv_r[:, ko, 0:128])
            for wct in range(1, 8):
                nc.sync.dma_start(
                    w_sb[:, :, wct * 128 : (wct + 1) * 128],
                    wqkv_r[:, :, wct * 128 : (wct + 1) * 128],
                )
            nc.sync.dma_start(cos_sb[:], cosT_d[:])
            nc.sync.dma_start(sinN_sb[:], sinN_d[:])
            nc.sync.dma_start(tril_sb[:], tril_d[:])
            nc.sync.dma_start(ident_sb[:], ident_d[:])
            for wct in range(8, CO_QKV // 128):
                nc.sync.dma_start(
                    w_sb[:, :, wct * 128 : (wct + 1) * 128],
                    wqkv_r[:, :, wct * 128 : (wct + 1) * 128],
                )
            nc.sync.dma_start(wp_sb[:], wproj_r[:])

            # ones columns of v (written once)
            nc.gpsimd.memset(v_sb[:, :, :, D], 1.0)

            # Filler queue: PE executes its queue in order, so attention
            # score passes (paced by ACT exp through the 2-deep score psum)
            # would stall PE. Interleave next-chunk QKV units and prior-chunk
            # projection tails between score tiles to keep PE fed.
            filler = []

            def emit_filler(n=1):
                for _ in range(min(n, len(filler))):
                    filler.pop(0)()

            for u in _phase_a_units(nc, 0, xtp, ropep, psA, xT_r, w_sb,
                                    cos_sb, sinN_sb, qT, kT, v_sb, xt0):
                u()
            for ic in range(N_IC):
                a_units = []
                if ic + 1 < N_IC:
                    a_units = _phase_a_units(
                        nc, ic + 1, xtp, ropep, psA, xT_r, w_sb,
                        cos_sb, sinN_sb, qT, kT, v_sb, xt0)
                filler.extend(a_units)
                tail_units = _phase_bc_chunk(
                    nc, ic, expp, finp, ysbp, ytrp, osp,
                    psA, psS, psY, qT, kT, v_sb, tril_sb,
                    ident_sb, wp_sb, out_d, emit_filler)
                # next BC chunk reads this chunk's q/k/v: flush leftovers
                emit_filler(len(filler))
                filler.extend(tail_units)
            emit_filler(len(filler))

    nc.compile()
    return nc


def _phase_a_units(nc, tc_i, xtp, ropep, psA, xT_r, w_sb, cos_sb, sinN_sb,
                   qT, kT, v_sb, xt0):
    """Return one closure per QKV unit (8 q/k column tiles + 4 v tiles)."""
    ts_ = slice(tc_i * TC, (tc_i + 1) * TC)
    if tc_i == 0:
        xt_sb = xt0
    else:
        xt_sb = xtp.tile([128, N_KO, TC], BF16, name="xt", tag="xt")
        for ko in range(N_KO):
            nc.sync.dma_start(xt_sb[:, ko], xT_r[:, ko, ts_])

    def qk_unit(ct):
        psq = psA.tile([128, TC], F32, name="psq", tag="pa")
        for ko in range(N_KO):
            nc.tensor.matmul(
                psq[:],
                w_sb[:, ko, ct * 128 : (ct + 1) * 128],
                xt_sb[:, ko, :],
                start=(ko == 0),
                stop=(ko == N_KO - 1),
            )
        hp = ct % 4
        dest = (qT if ct < 4 else kT)[:, hp, ts_]
        # RoPE: dest = q * cos + shift(q) * sinN, staged through SBUF bf16
        # (only DVE/ACT can read PSUM; bf16 SBUF ops run at 2-4x on DVE)
        qraw = ropep.tile([128, TC], BF16, name="qraw", tag="qraw")
        nc.vector.tensor_copy(qraw[:], psq[:])
        rot = ropep.tile([128, TC], BF16, name="rot", tag="rot")
        for blk in range(4):
            src = (blk ^ 1) * 32
            eng = nc.vector if blk % 2 == 0 else nc.gpsimd
            eng.tensor_copy(
                rot[blk * 32 : blk * 32 + 32, :],
                qraw[src : src + 32, :],
            )
        rs = ropep.tile([128, TC], BF16, name="rs", tag="rs")
        nc.vector.tensor_tensor(rs[:], rot[:], sinN_sb[:, ts_], OP.mult)
        t2 = ropep.tile([128, TC], BF16, name="t2", tag="t2")
        nc.vector.tensor_tensor(t2[:], qraw[:], cos_sb[:, ts_], OP.mult)
        nc.vector.tensor_tensor(dest, t2[:], rs[:], OP.add)

    def v_unit(sub):
        to = tc_i * (TC // 128) + sub
        psv = psA.tile([128, HPC * D], F32, name="psv", tag="pa")
        for ko in range(N_KO):
            nc.tensor.matmul(
                psv[:],
                xt_sb[:, ko, sub * 128 : sub * 128 + 128],
                w_sb[:, ko, 2 * HPC * D : 3 * HPC * D],
                start=(ko == 0),
                stop=(ko == N_KO - 1),
            )
        nc.vector.tensor_copy(
            v_sb[:, to, :, 0:D],
            psv[:].rearrange("p (h d) -> p h d", d=D),
        )

    units = [(lambda ct=ct: qk_unit(ct)) for ct in range(N_CT)]
    units += [(lambda sub=sub: v_unit(sub)) for sub in range(TC // 128)]
    return units


def _phase_bc_chunk(nc, ic, expp, finp, ysbp, ytrp, osp, psA, psS, psY,
                    qT, kT, v_sb, tril_sb, ident_sb, wp_sb, out_d,
                    emit_filler):
    n_jt = (ic + 1) * 4
    # per-qsub output accumulators are filled across (hp, hl); staged here
    y_q = [
        ysbp.tile([128, HPC, D], BF16, name=f"yq{qs}", tag="yq")
        for qs in range(4)
    ]
    def pass2_unit(hp, hl, qs, expts):
        # A@V oriented [queries, d+1], accumulate over key tiles
        qsg = ic * 4 + qs
        ypsum = psY.tile([128, TC], F32, name="ypsum", tag="ypsum")
        for jt in range(qsg + 1):
            expT, _lo = expts[jt]
            nc.tensor.matmul(
                ypsum[:, 0 : D + 1],
                expT[:, hl, qs * 128 : (qs + 1) * 128],
                v_sb[:, jt, 2 * hp + hl, :],
                start=(jt == 0),
                stop=(jt == qsg),
            )
        rcp = finp.tile([128, 1], F32, name="rcp", tag="rcp")
        nc.vector.reciprocal(rcp[:], ypsum[:, D : D + 1])
        nc.vector.tensor_scalar(
            y_q[qs][:, 2 * hp + hl, :],
            ypsum[:, 0:D],
            rcp[:],
            None,
            OP.mult,
        )

    # Each hp's AV pass is deferred into the next hp's ACT-paced score pass
    # as priority PE filler.
    pending2 = []
    for hp in range(HPC // 2):
        expts = []
        for jt in range(n_jt):
            k_diag = jt - ic * 4
            lo = max(0, k_diag * 128)
            sps = psS.tile([128, 2, TC], F32, name="sps", tag="sps")
            for hl in range(2):
                pb = hl * 64
                nc.tensor.matmul(
                    sps[:, hl, lo:],
                    kT[pb : pb + 64, hp, jt * 128 : (jt + 1) * 128],
                    qT[pb : pb + 64, hp, ic * IC + lo : (ic + 1) * IC],
                    start=True,
                    stop=True,
                )
            expT = expp.tile([128, 2, IC], BF16, name="expT", tag="expT")
            nc.scalar.activation(expT[:, :, lo:], sps[:, :, lo:], AF.Exp)
            if k_diag >= 0:
                # zero the upper-triangular part of the diagonal block
                nc.gpsimd.tensor_tensor(
                    expT[:, :, lo : lo + 128],
                    expT[:, :, lo : lo + 128],
                    tril_sb[:].rearrange("p (o c) -> p o c", o=1).broadcast_to(
                        [128, 2, 128]
                    ),
                    OP.mult,
                )
            expts.append((expT, lo))
            if pending2:
                pending2.pop(0)()
            else:
                emit_filler(1)
        for u in pending2:
            u()
        pending2 = [
            (lambda hp=hp, hl=hl, qs=qs, e=expts: pass2_unit(hp, hl, qs, e))
            for hl in range(2)
            for qs in range(4)
        ]
    for u in pending2:
        u()

    # tail: transpose y to [hd, q] and project (row-parallel partial).
    # Returned as filler closures so they slot into the next chunk's
    # ACT-paced score pass instead of serializing here.
    def tail_unit(qs):
        qt = ic * 4 + qs
        yT_sb = ytrp.tile([128, 4, 128], BF16, name="yT_sb", tag="yT_sb")
        for b in range(4):
            # each transpose owns a full psum bank (start=True zeroes 2KB)
            yT_ps = psA.tile([128, 1024], BF16, name="yT_ps", tag="pa")
            nc.tensor.transpose(
                yT_ps[:, 0:128],
                y_q[qs][:, 2 * b : 2 * b + 2, :].rearrange("p a d -> p (a d)"),
                ident_sb[:],
            )
            nc.vector.tensor_copy(yT_sb[:, b, :], yT_ps[:, 0:128])
        for cg in range(2):
            po = psA.tile([128, 512], F32, name="po", tag="pa")
            for b in range(4):
                nc.tensor.matmul(
                    po[:],
                    yT_sb[:, b, :],
                    wp_sb[:, b, cg * 512 : (cg + 1) * 512],
                    start=(b == 0),
                    stop=(b == 3),
                )
            ost = osp.tile([128, 512], F32)
            nc.vector.tensor_copy(ost[:], po[:])
            nc.sync.dma_start(
                out_d[qt * 128 : (qt + 1) * 128, cg * 512 : (cg + 1) * 512],
                ost[:],
            )

    return [(lambda qs=qs: tail_unit(qs)) for qs in range(4)]


_NC = None
_STATE = None


def _get_nc():
    global _NC
    if _NC is None:
        _NC = _build()
    return _NC


def _get_state():
    """Build the bass module once and cache a persistent jitted executor plus
    device-resident static tables (rope/mask/ones are pure functions of the
    problem shape)."""
    global _STATE
    if _STATE is not None:
        return _STATE

    import jax
    import jax.numpy as jnp
    from jax.experimental.shard_map import shard_map
    from jax.sharding import Mesh, NamedSharding, PartitionSpec

    from concourse import bass2jax

    nc = _get_nc()
    bass2jax.install_neuronx_cc_hook()
    partition_name = nc.partition_id_tensor.name if nc.partition_id_tensor else None
    in_names, out_names, out_avals = [], [], []
    for alloc in nc.m.functions[0].allocations:
        if not isinstance(alloc, mybir.MemoryLocationSet):
            continue
        name = alloc.memorylocations[0].name
        if alloc.kind == "ExternalInput":
            if name != partition_name:
                in_names.append(name)
        elif alloc.kind == "ExternalOutput":
            out_names.append(name)
            out_avals.append(
                jax.core.ShapedArray(tuple(alloc.tensor_shape), mybir.dt.np(alloc.dtype))
            )
    n_params, n_outs = len(in_names), len(out_avals)
    all_names = list(in_names) + out_names
    if partition_name:
        all_names.append(partition_name)

    def _body(*args):
        operands = list(args)
        if partition_name:
            operands.append(bass2jax.partition_id_tensor())
        outs = bass2jax._bass_exec_p.bind(
            *operands,
            out_avals=tuple(out_avals),
            in_names=tuple(all_names),
            out_names=tuple(out_names),
            lowering_input_output_aliases=(),
            sim_require_finite=True,
            sim_require_nnan=True,
            nc=nc,
        )
        return tuple(outs)

    devices = jax.devices()[:8]
    mesh = Mesh(np.asarray(devices), ("core",))
    shd = NamedSharding(mesh, PartitionSpec("core"))
    donate = tuple(range(n_params, n_params + n_outs))
    sharded = jax.jit(
        shard_map(
            _body,
            mesh=mesh,
            in_specs=(PartitionSpec("core"),) * (n_params + n_outs),
            out_specs=(PartitionSpec("core"),) * n_outs,
            check_rep=False,
        ),
        donate_argnums=donate,
        keep_unused=True,
    )
    zeros_fn = jax.jit(
        lambda: tuple(
            jnp.zeros((8 * av.shape[0],) + av.shape[1:], av.dtype) for av in out_avals
        ),
        out_shardings=(shd,) * n_outs,
    )

    cosT, sinN = _rope_tables()
    mask = _mask_table()
    ident = np.eye(128, dtype=np.float32)
    statics = {
        "cosT": jax.device_put(
            jnp.asarray(np.tile(cosT, (8, 1))).astype(jnp.bfloat16), shd
        ),
        "sinN": jax.device_put(
            jnp.asarray(np.tile(sinN, (8, 1))).astype(jnp.bfloat16), shd
        ),
        "tril01": jax.device_put(
            jnp.asarray(np.tile(mask, (8, 1))).astype(jnp.bfloat16), shd
        ),
        "ident": jax.device_put(
            jnp.asarray(np.tile(ident, (8, 1))).astype(jnp.bfloat16), shd
        ),
    }
    jax.block_until_ready(list(statics.values()))

    # On-device input dedup (x shared by core pairs, W by head-groups) and
    # output pair-reduction, as separate XLA modules.
    PAIRS = [[0, 1], [2, 3], [4, 5], [6, 7]]
    QUADS = [[0, 2, 4, 6], [1, 3, 5, 7]]

    def _pre(xs, wq, wp):
        xg = jax.lax.all_gather(xs, "core", axis_index_groups=PAIRS, axis=0, tiled=True)
        wqg = jax.lax.all_gather(wq, "core", axis_index_groups=QUADS, axis=0, tiled=True)
        wpg = jax.lax.all_gather(wp, "core", axis_index_groups=QUADS, axis=0, tiled=True)
        zeros = tuple(jnp.zeros(av.shape, av.dtype) for av in out_avals)
        return (
            xg.T.astype(jnp.bfloat16),
            wqg.astype(jnp.bfloat16),
            wpg.astype(jnp.bfloat16),
        ) + zeros

    pre_fn = jax.jit(
        shard_map(
            _pre,
            mesh=mesh,
            in_specs=(PartitionSpec("core"),) * 3,
            out_specs=(PartitionSpec("core"),) * (3 + n_outs),
        )
    )

    def _post(o):
        other = jax.lax.ppermute(
            o, "core", [(0, 1), (1, 0), (2, 3), (3, 2), (4, 5), (5, 4), (6, 7), (7, 6)]
        )
        s = o + other
        idx = jax.lax.axis_index("core")
        return jax.lax.dynamic_slice(s, ((idx % 2) * (T // 2), 0), (T // 2, C))

    post_fn = jax.jit(
        shard_map(
            _post,
            mesh=mesh,
            in_specs=(PartitionSpec("core"),),
            out_specs=PartitionSpec("core"),
        )
    )

    _STATE = dict(
        jax=jax,
        nc=nc,
        in_names=in_names,
        out_names=out_names,
        n_outs=n_outs,
        sharded=sharded,
        zeros_fn=zeros_fn,
        shd=shd,
        statics=statics,
        pre_fn=pre_fn,
        post_fn=post_fn,
    )
    return _STATE


def _prep_compact(x, W_attn, W_proj):
    """Compact (dedup'd) host inputs for the on-device gather pre-module."""
    xs = np.ascontiguousarray(x.reshape(8 * (T // 2), C))
    wqcat, wpcat = _prep_w(W_attn, W_proj)
    return xs, wqcat, wpcat


def _prep_w(W_attn, W_proj):
    scale = np.float32(1.0 / np.sqrt(D))
    wqkv_hg = []
    wproj_hg = []
    for hg in range(2):
        cs = slice(hg * HPC * D, (hg + 1) * HPC * D)
        wq = W_attn[:, 0 * C:][:, cs] * scale
        wk = W_attn[:, 1 * C:][:, cs]
        wv = W_attn[:, 2 * C:][:, cs]
        wqkv_hg.append(np.concatenate([wq, wk, wv], axis=1))
        wproj_hg.append(W_proj[cs, :])
    wqcat = np.empty((8 * 256, CO_QKV), dtype=np.float32)
    wpcat = np.empty((8 * 128, C), dtype=np.float32)
    for c in range(8):
        q = c // 2
        wqcat[c * 256 : (c + 1) * 256] = wqkv_hg[c % 2][q * 256 : (q + 1) * 256]
        wpcat[c * 128 : (c + 1) * 128] = wproj_hg[c % 2][q * 128 : (q + 1) * 128]
    return wqcat, wpcat


def _run_gathered(st, x, W_attn, W_proj):
    jax = st["jax"]
    # ship x first (async) so the transfer overlaps host-side W prep
    xs = np.ascontiguousarray(x.reshape(8 * (T // 2), C))
    d_xs = jax.device_put(xs, st["shd"])
    wqcat, wpcat = _prep_w(W_attn, W_proj)
    d_wq = jax.device_put(wqcat, st["shd"])
    d_wp = jax.device_put(wpcat, st["shd"])
    pre = st["pre_fn"](d_xs, d_wq, d_wp)
    dyn = {"xT": pre[0], "wqkv": pre[1], "wproj": pre[2]}
    args = [dyn[nm] if nm in dyn else st["statics"][nm] for nm in st["in_names"]]
    outs = st["sharded"](*args, *pre[3:])
    po = st["post_fn"](outs[0])
    r = np.asarray(po).reshape(B, T, C)
    return np.ascontiguousarray(r)


def _run_rbks(x, W_attn, W_proj):
    """Fallback: the stock run_bass_kernel_spmd entry point."""
    nc = _get_nc()
    cosT, sinN = _rope_tables()
    tril = _mask_table()
    bf = ml_dtypes.bfloat16

    def as_u16(a):
        return np.ascontiguousarray(a.astype(bf)).view(np.uint16)

    in_maps = []
    scale = np.float32(1.0 / np.sqrt(D))
    for core in range(8):
        b, hg = core // 2, core % 2
        cs = slice(hg * HPC * D, (hg + 1) * HPC * D)
        wq = W_attn[:, 0 * C:][:, cs] * scale
        wk = W_attn[:, 1 * C:][:, cs]
        wv = W_attn[:, 2 * C:][:, cs]
        in_maps.append(
            {
                "xT": as_u16(np.ascontiguousarray(x[b].T)),
                "wqkv": as_u16(np.concatenate([wq, wk, wv], axis=1)),
                "wproj": as_u16(W_proj[cs, :]),
                "cosT": as_u16(cosT),
                "sinN": as_u16(sinN),
                "tril01": as_u16(tril),
                "ident": as_u16(np.eye(128, dtype=np.float32)),
            }
        )
    res = run_bass_kernel_spmd(nc, in_maps, core_ids=list(range(8)))
    out = np.empty((B, T, C), dtype=np.float32)
    for b in range(B):
        out[b] = res.results[2 * b]["out"] + res.results[2 * b + 1]["out"]
    return out


def kernel(x, W_attn, W_proj):
    x = np.asarray(x, dtype=np.float32)
    W_attn = np.asarray(W_attn, dtype=np.float32)
    W_proj = np.asarray(W_proj, dtype=np.float32)

    try:
        st = _get_state()
        return _run_gathered(st, x, W_attn, W_proj)
    except Exception:
        return _run_rbks(x, W_attn, W_proj)


# revision 26
# speedup vs baseline: 1.1592x; 1.0425x over previous
"""Causal self-attention with RoPE on 8 Trainium2 NeuronCores.

Full inputs: x [4, 2048, 1024], W_attn [1024, 3072], W_proj [1024, 1024] (f32).
Sharding: core = b*2 + hg  (b in 0..3 batches, hg in 0..1 head-groups of 8 heads).
Each core computes qkv for its 8 heads, attention, and a partial output
projection (row-parallel c_proj); device pair-sum combines the two partials.

All matmuls run in bf16 (flat 1 cycle/row on the PE at any tile size, no
fp32r small-free penalty). Attention value matmul is oriented [queries, d]
with a fused ones-column denominator, normalized per-partition on DVE, then
PE-transposed for the bf16 row-parallel projection. Softmax exp is the only
Activation-engine work; phase A (QKV+RoPE) is interleaved per 512-token
chunk with attention so PE fills ACT-bound gaps.
"""

import sys

sys.path.insert(0, "/opt/trn_rl_repo")

import numpy as np
import ml_dtypes

import concourse.bass as bass  # noqa: F401
import concourse.mybir as mybir
import concourse.tile as tile
from concourse import bacc
from concourse.bass_utils import run_bass_kernel_spmd

F32 = mybir.dt.float32
BF16 = mybir.dt.bfloat16
AF = mybir.ActivationFunctionType
OP = mybir.AluOpType

B, T, C = 4, 2048, 1024
H, D = 16, 64
HPC = 8            # heads per core
CO_QKV = 3 * HPC * D   # 1536 qkv columns per core

TC = 512           # t-chunk width in phase A
N_TC = T // TC     # 4
N_KO = C // 128    # 8 contraction chunks
N_CT = 2 * HPC * D // 128   # 8 q+k column tiles (4 q, 4 k)
N_TT = T // 128    # 16 t tiles
IC = 512
N_IC = T // IC     # 4


def _rope_tables():
    """cosT f32 / sinN bf16 [128, T]: row p holds freq for d = p % 64; sinN has
    the rotate-half sign folded in (rows d<32 negative)."""
    inv_freq = (
        np.float32(1.0)
        / np.float32(10000.0) ** (np.arange(0, D, 2, dtype=np.float32) / np.float32(D))
    ).astype(np.float32)
    t = np.arange(T, dtype=np.float32)
    freqs = (t[:, None] * inv_freq[None, :]).astype(np.float32)  # [T, 32]
    emb = np.concatenate([freqs, freqs], axis=1)  # [T, 64]
    cos = np.cos(emb).astype(np.float32)
    sin = np.sin(emb).astype(np.float32)
    sinN = np.concatenate([-sin[:, :32], sin[:, 32:]], axis=1)
    cosT = np.tile(cos.T, (2, 1))   # [128, T]
    sinNT = np.tile(sinN.T, (2, 1))
    return np.ascontiguousarray(cosT), np.ascontiguousarray(sinNT)


def _mask_table():
    """tril01 [128, 128]: 1.0 if j <= c else 0.0 (key partition j, query col c)."""
    j = np.arange(128)[:, None]
    c = np.arange(128)[None, :]
    return (j <= c).astype(np.float32)


def _build():
    nc = bacc.Bacc(None, target_bir_lowering=False, debug=False)

    xT = nc.dram_tensor("xT", [C, T], BF16, kind="ExternalInput")
    wqkv = nc.dram_tensor("wqkv", [C, CO_QKV], BF16, kind="ExternalInput")
    wproj = nc.dram_tensor("wproj", [HPC * D, C], BF16, kind="ExternalInput")
    cosT_d = nc.dram_tensor("cosT", [128, T], BF16, kind="ExternalInput")
    sinN_d = nc.dram_tensor("sinN", [128, T], BF16, kind="ExternalInput")
    tril_d = nc.dram_tensor("tril01", [128, 128], BF16, kind="ExternalInput")
    ident_d = nc.dram_tensor("ident", [128, 128], BF16, kind="ExternalInput")
    out_d = nc.dram_tensor("out", [T, C], F32, kind="ExternalOutput")

    xT_r = xT.rearrange("(ko p) t -> p ko t", p=128)
    wqkv_r = wqkv.rearrange("(ko p) c -> p ko c", p=128)
    wproj_r = wproj.rearrange("(b p) c -> p b c", p=128)

    with tile.TileContext(nc) as tc:
        with (
            tc.tile_pool(name="resident", bufs=1) as res,
            tc.tile_pool(name="qkv", bufs=1) as qkv_pool,
            tc.tile_pool(name="xt", bufs=2) as xtp,
            tc.tile_pool(name="rope", bufs=5) as ropep,
            tc.tile_pool(name="exp", bufs=33) as expp,
            tc.tile_pool(name="fin", bufs=4) as finp,
            tc.tile_pool(name="ysb", bufs=9) as ysbp,
            tc.tile_pool(name="ytr", bufs=2) as ytrp,
            tc.tile_pool(name="ostage", bufs=2) as osp,
            tc.tile_pool(name="ps_a", bufs=2, space="PSUM") as psA,
            tc.tile_pool(name="ps_s", bufs=2, space="PSUM") as psS,
            tc.tile_pool(name="ps_y", bufs=2, space="PSUM") as psY,
        ):
            cos_sb = res.tile([128, T], BF16)
            sinN_sb = res.tile([128, T], BF16)
            tril_sb = res.tile([128, 128], BF16)
            ident_sb = res.tile([128, 128], BF16)
            w_sb = res.tile([128, N_KO, CO_QKV], BF16)
            wp_sb = res.tile([128, 4, C], BF16)

            # q^T / k^T: [p = d within head-pair, hp, t] bf16
            qT = qkv_pool.tile([128, HPC // 2, T], BF16)
            kT = qkv_pool.tile([128, HPC // 2, T], BF16)
            # v: [p = t%128, t//128, head, 65] with ones column at d=64
            v_sb = qkv_pool.tile([128, N_TT, HPC, D + 1], BF16)

            # ---- input DMA: first compute's inputs first ----
            xt0 = xtp.tile([128, N_KO, TC], BF16, name="xt0", tag="xt")
            for ko in range(N_KO):
                nc.sync.dma_start(xt0[:, ko], xT_r[:, ko, 0:TC])
                nc.sync.dma_start(w_sb[:, ko, 0:128], wqkv_r[:, ko, 0:128])
            for wct in range(1, 8):
                nc.sync.dma_start(
                    w_sb[:, :, wct * 128 : (wct + 1) * 128],
                    wqkv_r[:, :, wct * 128 : (wct + 1) * 128],
                )
            nc.sync.dma_start(cos_sb[:], cosT_d[:])
            nc.sync.dma_start(sinN_sb[:], sinN_d[:])
            nc.sync.dma_start(tril_sb[:], tril_d[:])
            nc.sync.dma_start(ident_sb[:], ident_d[:])
            for wct in range(8, CO_QKV // 128):
                nc.sync.dma_start(
                    w_sb[:, :, wct * 128 : (wct + 1) * 128],
                    wqkv_r[:, :, wct * 128 : (wct + 1) * 128],
                )
            nc.sync.dma_start(wp_sb[:], wproj_r[:])

            # ones columns of v (written once)
            nc.gpsimd.memset(v_sb[:, :, :, D], 1.0)

            # Filler queue: PE executes its queue in order, so attention
            # score passes (paced by ACT exp through the 2-deep score psum)
            # would stall PE. Interleave next-chunk QKV units and prior-chunk
            # projection tails between score tiles to keep PE fed.
            filler = []

            def emit_filler(n=1):
                for _ in range(min(n, len(filler))):
                    filler.pop(0)()

            for u in _phase_a_units(nc, 0, xtp, ropep, psA, xT_r, w_sb,
                                    cos_sb, sinN_sb, qT, kT, v_sb, xt0):
                u()
            for ic in range(N_IC):
                a_units = []
                if ic + 1 < N_IC:
                    a_units = _phase_a_units(
                        nc, ic + 1, xtp, ropep, psA, xT_r, w_sb,
                        cos_sb, sinN_sb, qT, kT, v_sb, xt0)
                filler.extend(a_units)
                tail_units = _phase_bc_chunk(
                    nc, ic, expp, finp, ysbp, ytrp, osp,
                    psA, psS, psY, qT, kT, v_sb, tril_sb,
                    ident_sb, wp_sb, out_d, emit_filler)
                # next BC chunk reads this chunk's q/k/v: flush leftovers
                emit_filler(len(filler))
                filler.extend(tail_units)
            emit_filler(len(filler))

    nc.compile()
    return nc


def _phase_a_units(nc, tc_i, xtp, ropep, psA, xT_r, w_sb, cos_sb, sinN_sb,
                   qT, kT, v_sb, xt0):
    """Return one closure per QKV unit (8 q/k column tiles + 4 v tiles)."""
    ts_ = slice(tc_i * TC, (tc_i + 1) * TC)
    if tc_i == 0:
        xt_sb = xt0
    else:
        xt_sb = xtp.tile([128, N_KO, TC], BF16, name="xt", tag="xt")
        for ko in range(N_KO):
            nc.sync.dma_start(xt_sb[:, ko], xT_r[:, ko, ts_])

    def qk_unit(ct):
        psq = psA.tile([128, TC], F32, name="psq", tag="pa")
        for ko in range(N_KO):
            nc.tensor.matmul(
                psq[:],
                w_sb[:, ko, ct * 128 : (ct + 1) * 128],
                xt_sb[:, ko, :],
                start=(ko == 0),
                stop=(ko == N_KO - 1),
            )
        hp = ct % 4
        dest = (qT if ct < 4 else kT)[:, hp, ts_]
        # RoPE: dest = q * cos + shift(q) * sinN, staged through SBUF bf16
        # (only DVE/ACT can read PSUM; bf16 SBUF ops run at 2-4x on DVE)
        qraw = ropep.tile([128, TC], BF16, name="qraw", tag="qraw")
        nc.vector.tensor_copy(qraw[:], psq[:])
        rot = ropep.tile([128, TC], BF16, name="rot", tag="rot")
        for blk in range(4):
            src = (blk ^ 1) * 32
            nc.vector.tensor_copy(
                rot[blk * 32 : blk * 32 + 32, :],
                qraw[src : src + 32, :],
            )
        rs = ropep.tile([128, TC], BF16, name="rs", tag="rs")
        nc.vector.tensor_tensor(rs[:], rot[:], sinN_sb[:, ts_], OP.mult)
        t2 = ropep.tile([128, TC], BF16, name="t2", tag="t2")
        nc.vector.tensor_tensor(t2[:], qraw[:], cos_sb[:, ts_], OP.mult)
        nc.vector.tensor_tensor(dest, t2[:], rs[:], OP.add)

    def v_unit(sub):
        to = tc_i * (TC // 128) + sub
        psv = psA.tile([128, HPC * D], F32, name="psv", tag="pa")
        for ko in range(N_KO):
            nc.tensor.matmul(
                psv[:],
                xt_sb[:, ko, sub * 128 : sub * 128 + 128],
                w_sb[:, ko, 2 * HPC * D : 3 * HPC * D],
                start=(ko == 0),
                stop=(ko == N_KO - 1),
            )
        nc.vector.tensor_copy(
            v_sb[:, to, :, 0:D],
            psv[:].rearrange("p (h d) -> p h d", d=D),
        )

    units = [(lambda ct=ct: qk_unit(ct)) for ct in range(N_CT)]
    units += [(lambda sub=sub: v_unit(sub)) for sub in range(TC // 128)]
    return units


def _phase_bc_chunk(nc, ic, expp, finp, ysbp, ytrp, osp, psA, psS, psY,
                    qT, kT, v_sb, tril_sb, ident_sb, wp_sb, out_d,
                    emit_filler):
    n_jt = (ic + 1) * 4
    # per-qsub output accumulators are filled across (hp, hl); staged here
    y_q = [
        ysbp.tile([128, HPC, D], BF16, name=f"yq{qs}", tag="yq")
        for qs in range(4)
    ]
    def pass2_unit(hp, hl, qs, expts):
        # A@V oriented [queries, d+1], accumulate over key tiles
        qsg = ic * 4 + qs
        ypsum = psY.tile([128, TC], F32, name="ypsum", tag="ypsum")
        for jt in range(qsg + 1):
            expT, _lo = expts[jt]
            nc.tensor.matmul(
                ypsum[:, 0 : D + 1],
                expT[:, hl, qs * 128 : (qs + 1) * 128],
                v_sb[:, jt, 2 * hp + hl, :],
                start=(jt == 0),
                stop=(jt == qsg),
            )
        rcp = finp.tile([128, 1], F32, name="rcp", tag="rcp")
        nc.vector.reciprocal(rcp[:], ypsum[:, D : D + 1])
        nc.vector.tensor_scalar(
            y_q[qs][:, 2 * hp + hl, :],
            ypsum[:, 0:D],
            rcp[:],
            None,
            OP.mult,
        )

    # Each hp's AV pass is deferred into the next hp's ACT-paced score pass
    # as priority PE filler.
    pending2 = []
    for hp in range(HPC // 2):
        expts = []
        for jt in range(n_jt):
            k_diag = jt - ic * 4
            lo = max(0, k_diag * 128)
            sps = psS.tile([128, 2, TC], F32, name="sps", tag="sps")
            for hl in range(2):
                pb = hl * 64
                nc.tensor.matmul(
                    sps[:, hl, lo:],
                    kT[pb : pb + 64, hp, jt * 128 : (jt + 1) * 128],
                    qT[pb : pb + 64, hp, ic * IC + lo : (ic + 1) * IC],
                    start=True,
                    stop=True,
                )
            expT = expp.tile([128, 2, IC], BF16, name="expT", tag="expT")
            nc.scalar.activation(expT[:, :, lo:], sps[:, :, lo:], AF.Exp)
            if k_diag >= 0:
                # zero the upper-triangular part of the diagonal block
                nc.gpsimd.tensor_tensor(
                    expT[:, :, lo : lo + 128],
                    expT[:, :, lo : lo + 128],
                    tril_sb[:].rearrange("p (o c) -> p o c", o=1).broadcast_to(
                        [128, 2, 128]
                    ),
                    OP.mult,
                )
            expts.append((expT, lo))
            if pending2:
                pending2.pop(0)()
            else:
                emit_filler(1)
        for u in pending2:
            u()
        pending2 = [
            (lambda hp=hp, hl=hl, qs=qs, e=expts: pass2_unit(hp, hl, qs, e))
            for hl in range(2)
            for qs in range(4)
        ]
    for u in pending2:
        u()

    # tail: transpose y to [hd, q] and project (row-parallel partial).
    # Returned as filler closures so they slot into the next chunk's
    # ACT-paced score pass instead of serializing here.
    def tail_unit(qs):
        qt = ic * 4 + qs
        yT_sb = ytrp.tile([128, 4, 128], BF16, name="yT_sb", tag="yT_sb")
        for b in range(4):
            # each transpose owns a full psum bank (start=True zeroes 2KB)
            yT_ps = psA.tile([128, 1024], BF16, name="yT_ps", tag="pa")
            nc.tensor.transpose(
                yT_ps[:, 0:128],
                y_q[qs][:, 2 * b : 2 * b + 2, :].rearrange("p a d -> p (a d)"),
                ident_sb[:],
            )
            nc.vector.tensor_copy(yT_sb[:, b, :], yT_ps[:, 0:128])
        for cg in range(2):
            po = psA.tile([128, 512], F32, name="po", tag="pa")
            for b in range(4):
                nc.tensor.matmul(
                    po[:],
                    yT_sb[:, b, :],
                    wp_sb[:, b, cg * 512 : (cg + 1) * 512],
                    start=(b == 0),
                    stop=(b == 3),
                )
            ost = osp.tile([128, 512], F32)
            nc.vector.tensor_copy(ost[:], po[:])
            nc.sync.dma_start(
                out_d[qt * 128 : (qt + 1) * 128, cg * 512 : (cg + 1) * 512],
                ost[:],
            )

    return [(lambda qs=qs: tail_unit(qs)) for qs in range(4)]


_NC = None
_STATE = None


def _get_nc():
    global _NC
    if _NC is None:
        _NC = _build()
    return _NC


def _get_state():
    """Build the bass module once and cache a persistent jitted executor plus
    device-resident static tables (rope/mask/ones are pure functions of the
    problem shape)."""
    global _STATE
    if _STATE is not None:
        return _STATE

    import jax
    import jax.numpy as jnp
    from jax.experimental.shard_map import shard_map
    from jax.sharding import Mesh, NamedSharding, PartitionSpec

    from concourse import bass2jax

    nc = _get_nc()
    bass2jax.install_neuronx_cc_hook()
    partition_name = nc.partition_id_tensor.name if nc.partition_id_tensor else None
    in_names, out_names, out_avals = [], [], []
    for alloc in nc.m.functions[0].allocations:
        if not isinstance(alloc, mybir.MemoryLocationSet):
            continue
        name = alloc.memorylocations[0].name
        if alloc.kind == "ExternalInput":
            if name != partition_name:
                in_names.append(name)
        elif alloc.kind == "ExternalOutput":
            out_names.append(name)
            out_avals.append(
                jax.core.ShapedArray(tuple(alloc.tensor_shape), mybir.dt.np(alloc.dtype))
            )
    n_params, n_outs = len(in_names), len(out_avals)
    all_names = list(in_names) + out_names
    if partition_name:
        all_names.append(partition_name)

    def _body(*args):
        operands = list(args)
        if partition_name:
            operands.append(bass2jax.partition_id_tensor())
        outs = bass2jax._bass_exec_p.bind(
            *operands,
            out_avals=tuple(out_avals),
            in_names=tuple(all_names),
            out_names=tuple(out_names),
            lowering_input_output_aliases=(),
            sim_require_finite=True,
            sim_require_nnan=True,
            nc=nc,
        )
        return tuple(outs)

    devices = jax.devices()[:8]
    mesh = Mesh(np.asarray(devices), ("core",))
    shd = NamedSharding(mesh, PartitionSpec("core"))
    donate = tuple(range(n_params, n_params + n_outs))
    sharded = jax.jit(
        shard_map(
            _body,
            mesh=mesh,
            in_specs=(PartitionSpec("core"),) * (n_params + n_outs),
            out_specs=(PartitionSpec("core"),) * n_outs,
            check_rep=False,
        ),
        donate_argnums=donate,
        keep_unused=True,
    )
    zeros_fn = jax.jit(
        lambda: tuple(
            jnp.zeros((8 * av.shape[0],) + av.shape[1:], av.dtype) for av in out_avals
        ),
        out_shardings=(shd,) * n_outs,
    )

    cosT, sinN = _rope_tables()
    mask = _mask_table()
    ident = np.eye(128, dtype=np.float32)
    statics = {
        "cosT": jax.device_put(
            jnp.asarray(np.tile(cosT, (8, 1))).astype(jnp.bfloat16), shd
        ),
        "sinN": jax.device_put(
            jnp.asarray(np.tile(sinN, (8, 1))).astype(jnp.bfloat16), shd
        ),
        "tril01": jax.device_put(
            jnp.asarray(np.tile(mask, (8, 1))).astype(jnp.bfloat16), shd
        ),
        "ident": jax.device_put(
            jnp.asarray(np.tile(ident, (8, 1))).astype(jnp.bfloat16), shd
        ),
    }
    jax.block_until_ready(list(statics.values()))

    # On-device input dedup (x shared by core pairs, W by head-groups) and
    # output pair-reduction, as separate XLA modules.
    PAIRS = [[0, 1], [2, 3], [4, 5], [6, 7]]
    QUADS = [[0, 2, 4, 6], [1, 3, 5, 7]]

    def _pre(xs, wq, wp):
        xg = jax.lax.all_gather(xs, "core", axis_index_groups=PAIRS, axis=0, tiled=True)
        wqg = jax.lax.all_gather(wq, "core", axis_index_groups=QUADS, axis=0, tiled=True)
        wpg = jax.lax.all_gather(wp, "core", axis_index_groups=QUADS, axis=0, tiled=True)
        zeros = tuple(jnp.zeros(av.shape, av.dtype) for av in out_avals)
        return (
            xg.T.astype(jnp.bfloat16),
            wqg.astype(jnp.bfloat16),
            wpg.astype(jnp.bfloat16),
        ) + zeros

    pre_fn = jax.jit(
        shard_map(
            _pre,
            mesh=mesh,
            in_specs=(PartitionSpec("core"),) * 3,
            out_specs=(PartitionSpec("core"),) * (3 + n_outs),
        )
    )

    def _post(o):
        other = jax.lax.ppermute(
            o, "core", [(0, 1), (1, 0), (2, 3), (3, 2), (4, 5), (5, 4), (6, 7), (7, 6)]
        )
        s = o + other
        idx = jax.lax.axis_index("core")
        return jax.lax.dynamic_slice(s, ((idx % 2) * (T // 2), 0), (T // 2, C))

    post_fn = jax.jit(
        shard_map(
            _post,
            mesh=mesh,
            in_specs=(PartitionSpec("core"),),
            out_specs=PartitionSpec("core"),
        )
    )

    _STATE = dict(
        jax=jax,
        nc=nc,
        in_names=in_names,
        out_names=out_names,
        n_outs=n_outs,
        sharded=sharded,
        zeros_fn=zeros_fn,
        shd=shd,
        statics=statics,
        pre_fn=pre_fn,
        post_fn=post_fn,
    )
    return _STATE


def _prep_compact(x, W_attn, W_proj):
    """Compact (dedup'd) host inputs for the on-device gather pre-module."""
    xs = np.ascontiguousarray(x.reshape(8 * (T // 2), C))
    wqcat, wpcat = _prep_w(W_attn, W_proj)
    return xs, wqcat, wpcat


def _prep_w(W_attn, W_proj):
    scale = np.float32(1.0 / np.sqrt(D))
    wqkv_hg = []
    wproj_hg = []
    for hg in range(2):
        cs = slice(hg * HPC * D, (hg + 1) * HPC * D)
        wq = W_attn[:, 0 * C:][:, cs] * scale
        wk = W_attn[:, 1 * C:][:, cs]
        wv = W_attn[:, 2 * C:][:, cs]
        wqkv_hg.append(np.concatenate([wq, wk, wv], axis=1))
        wproj_hg.append(W_proj[cs, :])
    wqcat = np.empty((8 * 256, CO_QKV), dtype=np.float32)
    wpcat = np.empty((8 * 128, C), dtype=np.float32)
    for c in range(8):
        q = c // 2
        wqcat[c * 256 : (c + 1) * 256] = wqkv_hg[c % 2][q * 256 : (q + 1) * 256]
        wpcat[c * 128 : (c + 1) * 128] = wproj_hg[c % 2][q * 128 : (q + 1) * 128]
    return wqcat, wpcat


def _run_gathered(st, x, W_attn, W_proj):
    jax = st["jax"]
    # ship x first (async) so the transfer overlaps host-side W prep
    xs = np.ascontiguousarray(x.reshape(8 * (T // 2), C))
    d_xs = jax.device_put(xs, st["shd"])
    wqcat, wpcat = _prep_w(W_attn, W_proj)
    d_wq = jax.device_put(wqcat, st["shd"])
    d_wp = jax.device_put(wpcat, st["shd"])
    pre = st["pre_fn"](d_xs, d_wq, d_wp)
    dyn = {"xT": pre[0], "wqkv": pre[1], "wproj": pre[2]}
    args = [dyn[nm] if nm in dyn else st["statics"][nm] for nm in st["in_names"]]
    outs = st["sharded"](*args, *pre[3:])
    po = st["post_fn"](outs[0])
    r = np.asarray(po).reshape(B, T, C)
    return np.ascontiguousarray(r)


def _run_rbks(x, W_attn, W_proj):
    """Fallback: the stock run_bass_kernel_spmd entry point."""
    nc = _get_nc()
    cosT, sinN = _rope_tables()
    tril = _mask_table()
    bf = ml_dtypes.bfloat16

    def as_u16(a):
        return np.ascontiguousarray(a.astype(bf)).view(np.uint16)

    in_maps = []
    scale = np.float32(1.0 / np.sqrt(D))
    for core in range(8):
        b, hg = core // 2, core % 2
        cs = slice(hg * HPC * D, (hg + 1) * HPC * D)
        wq = W_attn[:, 0 * C:][:, cs] * scale
        wk = W_attn[:, 1 * C:][:, cs]
        wv = W_attn[:, 2 * C:][:, cs]
        in_maps.append(
            {
                "xT": as_u16(np.ascontiguousarray(x[b].T)),
                "wqkv": as_u16(np.concatenate([wq, wk, wv], axis=1)),
                "wproj": as_u16(W_proj[cs, :]),
                "cosT": as_u16(cosT),
                "sinN": as_u16(sinN),
                "tril01": as_u16(tril),
                "ident": as_u16(np.eye(128, dtype=np.float32)),
            }
        )
    res = run_bass_kernel_spmd(nc, in_maps, core_ids=list(range(8)))
    out = np.empty((B, T, C), dtype=np.float32)
    for b in range(B):
        out[b] = res.results[2 * b]["out"] + res.results[2 * b + 1]["out"]
    return out


def kernel(x, W_attn, W_proj):
    x = np.asarray(x, dtype=np.float32)
    W_attn = np.asarray(W_attn, dtype=np.float32)
    W_proj = np.asarray(W_proj, dtype=np.float32)

    try:
        st = _get_state()
        return _run_gathered(st, x, W_attn, W_proj)
    except Exception:
        return _run_rbks(x, W_attn, W_proj)
